# revision 11
# baseline (speedup 1.0000x reference)
"""Trainium2 Bass kernel for the nn_AttentionLayer problem.

Full multi-head attention layer, B=4, L=S=2048, d_model=1024, 16 heads of
dim 64, with the reference's "mix=True" transpose-then-flatten before the
output projection.

Key observation: the mix reshape means output row l' = h*128 + l//16 of each
batch depends ONLY on head h.  So sharding 8 cores as (batch, half-of-heads)
makes every core produce a disjoint, contiguous 1024-row slice of the output
with zero cross-core communication.

Per-core dataflow:
  - inputs arrive host-transposed ([D, L]) so the QKV projections contract
    d_model on the partition dim with natural weight layouts
  - q/k projections produce qT/kT [e, l] (heads on partitions, pairs of
    heads per 128-partition tile), with 1/sqrt(64) and bias folded in
  - scoresT tile [s=128, l=512] = kT.T @ qT per (s-tile, l-chunk); exp on
    the scalar engine; A@V accumulates v_aug = [v | ones] stationary so
    row 64 of the accumulator is the softmax denominator
  - normalize with vector-reciprocal + gpsimd partition_broadcast
  - the normalized attn [64, L] is self-copied (SBUF->SBUF DMA) to
    partitions 64..127 shifted by one position, which makes the output
    projection a clean K=128 matmul against natural Wo row-pair tiles

All matmuls run in `mm_dt` (float32r by default: full-rate fp32 on the PE).
"""

import numpy as np

import concourse.bass as bass
import concourse.mybir as mybir
import concourse.tile as tile
from concourse import bacc
from concourse.bass_utils import run_bass_kernel_spmd

F32 = mybir.dt.float32
EXP = mybir.ActivationFunctionType.Exp
MULT = mybir.AluOpType.mult
ADD = mybir.AluOpType.add

E = 64          # head dim
J = 16          # mix factor: total heads in the reference model
JE = J * E      # 1024 rows of Wo

MM_DTS = {
    "f32r": mybir.dt.float32r,
    "f32": mybir.dt.float32,
    "bf16": mybir.dt.bfloat16,
}


def build_core_kernel(L=2048, D=1024, NH=8, OUT_D=1024, mm_dt="f32r"):
    """Builds the per-core Bacc graph (SPMD: all 8 cores run this)."""
    HE = NH * E               # projected width per core
    HEC = HE // 128           # qT/kT tiles (head pairs)
    NHP = NH // 2             # head pairs
    KC = D // 128             # contraction tiles for projections
    R = L // J                # output rows per head
    LCH = min(512, L)         # l-chunk
    NLC = L // LCH
    NST = L // 128            # s-tiles
    DCH = min(512, OUT_D)     # out-proj n-chunk
    NDC = OUT_D // DCH
    SCALE = 1.0 / np.sqrt(E)
    assert L % J == 0 and R <= 128 and HE % 128 == 0

    MDT = MM_DTS[mm_dt]

    nc = bacc.Bacc("TRN2", target_bir_lowering=False, debug=False,
                   enable_asserts=False)

    qT_ext = nc.declare_dram_parameter("qT", [D, L], MDT, isOutput=False)
    kT_ext = nc.declare_dram_parameter("kT", [D, L], MDT, isOutput=False)
    vT_ext = nc.declare_dram_parameter("vT", [D, L], MDT, isOutput=False)
    wq_ext = nc.declare_dram_parameter("wq", [D, HE], MDT, isOutput=False)
    wk_ext = nc.declare_dram_parameter("wk", [D, HE], MDT, isOutput=False)
    wv_ext = nc.declare_dram_parameter("wv", [D, HE], MDT, isOutput=False)
    bq_ext = nc.declare_dram_parameter("bq", [HE], F32, isOutput=False)
    bk_ext = nc.declare_dram_parameter("bk", [HE], F32, isOutput=False)
    bv_ext = nc.declare_dram_parameter("bv", [HE], F32, isOutput=False)
    wo_ext = nc.declare_dram_parameter("wo", [JE, OUT_D], MDT, isOutput=False)
    bo_ext = nc.declare_dram_parameter("bo", [OUT_D], F32, isOutput=False)
    out_ext = nc.declare_dram_parameter("out", [NH * R, OUT_D], F32,
                                        isOutput=True)

    with tile.TileContext(nc) as tc:
        with (
            tc.tile_pool(name="const", bufs=1) as const,
            tc.tile_pool(name="wsl", bufs=min(KC + 1, 3 * KC)) as wsl,
            tc.tile_pool(name="qin", bufs=3) as qin,
            tc.tile_pool(name="acts", bufs=1) as acts,
            tc.tile_pool(name="expp", bufs=4) as expp,
            tc.tile_pool(name="attnd", bufs=2) as attnd,
            tc.tile_pool(name="small", bufs=2) as small,
            tc.tile_pool(name="outp", bufs=4) as outp,
        ):
            # ---- constants ----
            bqt = const.tile([128, HEC], F32, tag="bqt")
            nc.sync.dma_start(bqt[:], bq_ext.rearrange("(c p) -> p c", p=128))
            bqs = const.tile([128, HEC], F32, tag="bqs")
            nc.vector.tensor_scalar_mul(bqs[:], bqt[:], float(SCALE))
            bkt = const.tile([128, HEC], F32, tag="bkt")
            nc.sync.dma_start(bkt[:], bk_ext.rearrange("(c p) -> p c", p=128))

            bv_row = const.tile([1, HE], F32, tag="bv_row")
            nc.sync.dma_start(bv_row[:],
                              bv_ext.rearrange("(o he) -> o he", o=1))
            bv_bc = const.tile([128, HE], F32, tag="bv_bc")
            nc.gpsimd.partition_broadcast(bv_bc[:], bv_row[:], channels=128)

            bo_row = const.tile([1, OUT_D], F32, tag="bo_row")
            nc.sync.dma_start(bo_row[:],
                              bo_ext.rearrange("(o d) -> o d", o=1))
            bo_bc = const.tile([128, OUT_D], F32, tag="bo_bc")
            nc.gpsimd.partition_broadcast(bo_bc[:], bo_row[:], channels=128)

            ones_t = const.tile([128, NH], F32, tag="ones_t")
            nc.vector.memset(ones_t[:], 1.0)

            wo_sb = []
            for t in range(JE // 128):
                w = const.tile([128, OUT_D], MDT, tag=f"wo{t}",
                               name=f"wo_sb{t}")
                nc.sync.dma_start(w[:], wo_ext[t * 128:(t + 1) * 128, :])
                wo_sb.append(w)

            # ---- phase 1: q/k projections -> qT/kT [e, l] head-pair tiles
            qT_sb = [acts.tile([128, L], MDT, tag=f"qT{i}", name=f"qT_sb{i}")
                     for i in range(HEC)]
            kT_sb = [acts.tile([128, L], MDT, tag=f"kT{i}", name=f"kT_sb{i}")
                     for i in range(HEC)]

            pp_ctx = tc.tile_pool(name="pp", bufs=min(HEC + 2, 6),
                                  space="PSUM")
            pp = pp_ctx.__enter__()

            for which, w_ext, in_ext, dest in (
                ("q", wq_ext, qT_ext, qT_sb),
                ("k", wk_ext, kT_ext, kT_sb),
            ):
                wt = []
                for dt in range(KC):
                    w = wsl.tile([128, HE], MDT, tag="wsl", name="w_t")
                    nc.sync.dma_start(w[:], w_ext[dt * 128:(dt + 1) * 128, :])
                    wt.append(w)
                for lc in range(NLC):
                    ps = [pp.tile([128, LCH], F32, tag="pp", name=f"pp{i}")
                          for i in range(HEC)]
                    for dt in range(KC):
                        xin = qin.tile([128, LCH], MDT, tag="qin")
                        nc.sync.dma_start(
                            xin[:],
                            in_ext[dt * 128:(dt + 1) * 128,
                                   lc * LCH:(lc + 1) * LCH])
                        for ec in range(HEC):
                            nc.tensor.matmul(
                                ps[ec][:],
                                wt[dt][:, ec * 128:(ec + 1) * 128],
                                xin[:],
                                start=(dt == 0), stop=(dt == KC - 1))
                    for ec in range(HEC):
                        dst = dest[ec][:, lc * LCH:(lc + 1) * LCH]
                        if which == "q":
                            # (psum + bq) * scale
                            nc.vector.tensor_scalar(
                                dst, ps[ec][:], float(SCALE),
                                bqs[:, ec:ec + 1], MULT, ADD)
                        else:
                            nc.vector.tensor_scalar(
                                dst, ps[ec][:], bkt[:, ec:ec + 1], None, ADD)

            # ---- phase 1b: v projection -> v_aug [s, NH*(E+1)] s-tiles
            wvt = []
            for dt in range(KC):
                w = wsl.tile([128, HE], MDT, tag="wsl", name="w_t")
                nc.sync.dma_start(w[:], wv_ext[dt * 128:(dt + 1) * 128, :])
                wvt.append(w)
            v_aug = []
            for st in range(NST):
                v = acts.tile([128, NH * (E + 1)], MDT, tag=f"vaug{st}",
                              name=f"vaug{st}")
                v_aug.append(v)
                nc.vector.tensor_copy(
                    v.rearrange("p (h u) -> p h u", u=E + 1)[:, :, E:E + 1],
                    ones_t.rearrange("p (h o) -> p h o", o=1))
                psv = pp.tile([128, HE], F32, tag="pp", name="psv")
                for dt in range(KC):
                    vin = qin.tile([128, 128], MDT, tag="vin")
                    nc.sync.dma_start(
                        vin[:],
                        vT_ext[dt * 128:(dt + 1) * 128,
                               st * 128:(st + 1) * 128])
                    nc.tensor.matmul(psv[:], vin[:], wvt[dt][:],
                                     start=(dt == 0), stop=(dt == KC - 1))
                nc.vector.tensor_add(
                    v.rearrange("p (h u) -> p h u", u=E + 1)[:, :, 0:E],
                    psv.rearrange("p (h e) -> p h e", e=E)[:],
                    bv_bc.rearrange("p (h e) -> p h e", e=E)[:])

            pp_ctx.__exit__(None, None, None)

            # ---- phase 2+3: attention + output projection per head pair
            with (
                tc.tile_pool(name="psc", bufs=4, space="PSUM") as psc,
                tc.tile_pool(name="pav", bufs=2, space="PSUM") as pav,
                tc.tile_pool(name="pout", bufs=2, space="PSUM") as pout,
            ):
                for hp in range(NHP):
                    dups = []
                    for loc in range(2):  # head A (partitions 0:64), head B
                        p0 = loc * 64
                        dup = attnd.tile([128, L], MDT, tag="attnd",
                                         name="dup")
                        dups.append(dup)
                        for lc in range(NLC):
                            pavx = pav.tile([65, LCH], F32, tag="pav",
                                            name="pavx")
                            for st in range(NST):
                                sc = psc.tile([128, LCH], F32, tag="psc",
                                              name="sc")
                                nc.tensor.matmul(
                                    sc[:],
                                    kT_sb[hp][p0:p0 + 64,
                                              st * 128:(st + 1) * 128],
                                    qT_sb[hp][p0:p0 + 64,
                                              lc * LCH:(lc + 1) * LCH],
                                    start=True, stop=True)
                                ex = expp.tile([128, LCH], MDT, tag="exp",
                                               name="ex")
                                nc.scalar.activation(ex[:], sc[:], EXP)
                                h = 2 * hp + loc
                                nc.tensor.matmul(
                                    pavx[:],
                                    v_aug[st][:, h * (E + 1):
                                              (h + 1) * (E + 1)],
                                    ex[:],
                                    start=(st == 0), stop=(st == NST - 1))
                            rc = small.tile([1, LCH], F32, tag="rc",
                                            name="rc")
                            nc.vector.reciprocal(rc[:], pavx[64:65, :])
                            bc = small.tile([64, LCH], F32, tag="bc",
                                            name="bc")
                            nc.gpsimd.partition_broadcast(bc[:], rc[:],
                                                          channels=64)
                            nc.vector.tensor_mul(
                                dup[0:64, lc * LCH:(lc + 1) * LCH],
                                pavx[0:64, :], bc[:])
                        # shifted self-copy: partition 64+e, col l holds
                        # attn[e, l+1]; a stride-16 AP at offset 2t then
                        # reads the (2t, 2t+1) j-pair as one K=128
                        # stationary operand for the output projection
                        nc.sync.dma_start(dup[64:128, 0:L - 1],
                                          dup[0:64, 1:L])

                    for loc in range(2):
                        h = 2 * hp + loc
                        dup = dups[loc]
                        lhs = dup.rearrange("p (r j) -> p j r", j=J)
                        for dc in range(NDC):
                            po = pout.tile([R, DCH], F32, tag="pout",
                                           name="po")
                            for t in range(JE // 128):
                                nc.tensor.matmul(
                                    po[:],
                                    lhs[:, 2 * t, :],
                                    wo_sb[t][:, dc * DCH:(dc + 1) * DCH],
                                    start=(t == 0),
                                    stop=(t == JE // 128 - 1))
                            ob = outp.tile([R, DCH], F32, tag="outp",
                                           name="ob")
                            nc.vector.tensor_add(
                                ob[:], po[:],
                                bo_bc[0:R, dc * DCH:(dc + 1) * DCH])
                            nc.sync.dma_start(
                                out_ext[h * R:(h + 1) * R,
                                        dc * DCH:(dc + 1) * DCH],
                                ob[:])

    nc.compile()
    return nc


# ---------------------------------------------------------------------------
# host side
# ---------------------------------------------------------------------------

_NC_CACHE = {}

FULL_KEY = (2048, 1024, 8, 1024, "f32r")


def _get_nc(key=FULL_KEY):
    if key not in _NC_CACHE:
        _NC_CACHE[key] = build_core_kernel(*key)
    return _NC_CACHE[key]


def _np_mm_dtype(mm_dt):
    if mm_dt == "bf16":
        import ml_dtypes
        return ml_dtypes.bfloat16
    return np.float32


def make_in_maps(queries, keys, values, Wq, bq, Wk, bk, Wv, bv, Wo, bo,
                 mm_dt="f32r"):
    """Shard: core c handles batch c//2, heads NH*(c%2) .. NH*(c%2)+NH."""
    f = np.float32
    md = _np_mm_dtype(mm_dt)
    half_w = np.asarray(Wq).shape[1] // 2
    in_maps = []
    for c in range(8):
        b, half = c // 2, c % 2
        cs = slice(half * half_w, (half + 1) * half_w)
        in_maps.append({
            "qT": np.ascontiguousarray(np.asarray(queries[b], f).T.astype(md)),
            "kT": np.ascontiguousarray(np.asarray(keys[b], f).T.astype(md)),
            "vT": np.ascontiguousarray(np.asarray(values[b], f).T.astype(md)),
            "wq": np.ascontiguousarray(np.asarray(Wq, f)[:, cs].astype(md)),
            "wk": np.ascontiguousarray(np.asarray(Wk, f)[:, cs].astype(md)),
            "wv": np.ascontiguousarray(np.asarray(Wv, f)[:, cs].astype(md)),
            "bq": np.ascontiguousarray(np.asarray(bq, f)[cs]),
            "bk": np.ascontiguousarray(np.asarray(bk, f)[cs]),
            "bv": np.ascontiguousarray(np.asarray(bv, f)[cs]),
            "wo": np.ascontiguousarray(np.asarray(Wo, f).astype(md)),
            "bo": np.ascontiguousarray(np.asarray(bo, f)),
        })
    return in_maps


def assemble_output(results, B=4, L=2048, OUT_D=1024):
    out = np.empty((B, L, OUT_D), np.float32)
    half_rows = L // 2
    for c in range(8):
        b, half = c // 2, c % 2
        out[b, half * half_rows:(half + 1) * half_rows, :] = results[c]["out"]
    return out


def run_on_hw(inputs, trace=False, key=FULL_KEY, **kw):
    nc = _get_nc(key)
    in_maps = make_in_maps(**inputs, mm_dt=key[4])
    res = run_bass_kernel_spmd(nc, in_maps, core_ids=list(range(8)),
                               trace=trace, **kw)
    return assemble_output(res.results), res


def kernel(**inputs) -> np.ndarray:
    out, _ = run_on_hw(inputs, trace=False)
    return out


# revision 21
# speedup vs baseline: 1.2327x; 1.2327x over previous
"""Trainium2 Bass kernel for the nn_AttentionLayer problem.

Full multi-head attention layer, B=4, L=S=2048, d_model=1024, 16 heads of
dim 64, with the reference's "mix=True" transpose-then-flatten before the
output projection.

Key observation: the mix reshape means output row l' = h*128 + l//16 of each
batch depends ONLY on head h.  So sharding 8 cores as (batch, half-of-heads)
makes every core produce a disjoint, contiguous 1024-row slice of the output
with zero cross-core communication.

Per-core dataflow:
  - inputs arrive host-transposed ([D, L]) so the QKV projections contract
    d_model on the partition dim with natural weight layouts
  - q/k projections produce qT/kT [e, l] (heads on partitions, pairs of
    heads per 128-partition tile), with 1/sqrt(64) and bias folded in
  - scoresT tile [s=128, l=512] = kT.T @ qT per (s-tile, l-chunk); exp on
    the scalar engine; A@V accumulates v_aug = [v | ones] stationary so
    row 64 of the accumulator is the softmax denominator
  - normalize with vector-reciprocal + gpsimd partition_broadcast
  - the normalized attn [64, L] is self-copied (SBUF->SBUF DMA) to
    partitions 64..127 shifted by one position, which makes the output
    projection a clean K=128 matmul against natural Wo row-pair tiles

All matmuls run in `mm_dt` (float32r by default: full-rate fp32 on the PE).
"""

import numpy as np

import concourse.bass as bass
import concourse.mybir as mybir
import concourse.tile as tile
from concourse import bacc
from concourse.bass_utils import run_bass_kernel_spmd

F32 = mybir.dt.float32
EXP = mybir.ActivationFunctionType.Exp
MULT = mybir.AluOpType.mult
ADD = mybir.AluOpType.add

E = 64          # head dim
J = 16          # mix factor: total heads in the reference model
JE = J * E      # 1024 rows of Wo

MM_DTS = {
    "f32r": mybir.dt.float32r,
    "f32": mybir.dt.float32,
    "bf16": mybir.dt.bfloat16,
}


def build_core_kernel(L=2048, D=1024, NH=8, OUT_D=1024, mm_dt="f32r",
                      taps=False):
    """Builds the per-core Bacc graph (SPMD: all 8 cores run this)."""
    HE = NH * E               # projected width per core
    HEC = HE // 128           # qT/kT tiles (head pairs)
    NHP = NH // 2             # head pairs
    KC = D // 128             # contraction tiles for projections
    R = L // J                # output rows per head
    LCH = min(512, L)         # l-chunk
    NLC = L // LCH
    NST = L // 128            # s-tiles
    DCH = min(512, OUT_D)     # out-proj n-chunk
    NDC = OUT_D // DCH
    SCALE = 1.0 / np.sqrt(E)
    assert L % J == 0 and R <= 128 and HE % 128 == 0

    MDT = MM_DTS[mm_dt]

    nc = bacc.Bacc("TRN2", target_bir_lowering=False, debug=False,
                   enable_asserts=False)

    qT_ext = nc.declare_dram_parameter("qT", [D, L], MDT, isOutput=False)
    kT_ext = nc.declare_dram_parameter("kT", [D, L], MDT, isOutput=False)
    vT_ext = nc.declare_dram_parameter("vT", [D, L], MDT, isOutput=False)
    wq_ext = nc.declare_dram_parameter("wq", [D, HE], MDT, isOutput=False)
    wk_ext = nc.declare_dram_parameter("wk", [D, HE], MDT, isOutput=False)
    wv_ext = nc.declare_dram_parameter("wv", [D, HE], MDT, isOutput=False)
    bq_ext = nc.declare_dram_parameter("bq", [HE], F32, isOutput=False)
    bk_ext = nc.declare_dram_parameter("bk", [HE], F32, isOutput=False)
    bv_ext = nc.declare_dram_parameter("bv", [HE], F32, isOutput=False)
    wo_ext = nc.declare_dram_parameter("wo", [JE, OUT_D], MDT, isOutput=False)
    bo_ext = nc.declare_dram_parameter("bo", [OUT_D], F32, isOutput=False)
    out_ext = nc.declare_dram_parameter("out", [NH * R, OUT_D], F32,
                                        isOutput=True)
    HEC_ = HE // 128
    if taps:
        dbg_qT = nc.declare_dram_parameter("dbg_qT", [HEC_ * 128, L], MDT,
                                           isOutput=True)
        dbg_kT = nc.declare_dram_parameter("dbg_kT", [HEC_ * 128, L], MDT,
                                           isOutput=True)
        dbg_v = nc.declare_dram_parameter("dbg_v", [(L // 128) * 128,
                                                    NH * (E + 1)], MDT,
                                          isOutput=True)
        dbg_ex = nc.declare_dram_parameter("dbg_ex", [128, min(512, L)], MDT,
                                           isOutput=True)
        dbg_dup = nc.declare_dram_parameter("dbg_dup", [128, L], MDT,
                                            isOutput=True)

    with tile.TileContext(nc) as tc:
        with (
            tc.tile_pool(name="const", bufs=1) as const,
            tc.tile_pool(name="wsl", bufs=min(KC + 1, 3 * KC)) as wsl,
            tc.tile_pool(name="qin", bufs=3) as qin,
            tc.tile_pool(name="acts", bufs=1) as acts,
            tc.tile_pool(name="expp", bufs=4) as expp,
            tc.tile_pool(name="attnd", bufs=2) as attnd,
            tc.tile_pool(name="small", bufs=2) as small,
            tc.tile_pool(name="outp", bufs=4) as outp,
        ):
            # ---- constants ----
            bqt = const.tile([128, HEC], F32, tag="bqt")
            nc.sync.dma_start(bqt[:], bq_ext.rearrange("(c p) -> p c", p=128))
            bqs = const.tile([128, HEC], F32, tag="bqs")
            nc.vector.tensor_scalar_mul(bqs[:], bqt[:], float(SCALE))
            bkt = const.tile([128, HEC], F32, tag="bkt")
            nc.sync.dma_start(bkt[:], bk_ext.rearrange("(c p) -> p c", p=128))

            bv_row = const.tile([1, HE], F32, tag="bv_row")
            nc.sync.dma_start(bv_row[:],
                              bv_ext.rearrange("(o he) -> o he", o=1))
            bv_bc = const.tile([128, HE], F32, tag="bv_bc")
            nc.gpsimd.partition_broadcast(bv_bc[:], bv_row[:], channels=128)

            bo_row = const.tile([1, OUT_D], F32, tag="bo_row")
            nc.sync.dma_start(bo_row[:],
                              bo_ext.rearrange("(o d) -> o d", o=1))
            bo_bc = const.tile([128, OUT_D], F32, tag="bo_bc")
            nc.gpsimd.partition_broadcast(bo_bc[:], bo_row[:], channels=128)

            ones_t = const.tile([128, NH], F32, tag="ones_t")
            nc.vector.memset(ones_t[:], 1.0)

            wo_sb = []
            for t in range(JE // 128):
                w = const.tile([128, OUT_D], MDT, tag=f"wo{t}",
                               name=f"wo_sb{t}")
                nc.sync.dma_start(w[:], wo_ext[t * 128:(t + 1) * 128, :])
                wo_sb.append(w)

            # ---- phase 1: q/k projections -> qT/kT [e, l] head-pair tiles
            qT_sb = [acts.tile([128, L], MDT, tag=f"qT{i}", name=f"qT_sb{i}")
                     for i in range(HEC)]
            kT_sb = [acts.tile([128, L], MDT, tag=f"kT{i}", name=f"kT_sb{i}")
                     for i in range(HEC)]

            pp_ctx = tc.tile_pool(name="pp", bufs=min(HEC + 2, 6),
                                  space="PSUM")
            pp = pp_ctx.__enter__()

            for which, w_ext, in_ext, dest in (
                ("q", wq_ext, qT_ext, qT_sb),
                ("k", wk_ext, kT_ext, kT_sb),
            ):
                wt = []
                for dt in range(KC):
                    w = wsl.tile([128, HE], MDT, tag="wsl", name="w_t")
                    nc.sync.dma_start(w[:], w_ext[dt * 128:(dt + 1) * 128, :])
                    wt.append(w)
                for lc in range(NLC):
                    ps = [pp.tile([128, LCH], F32, tag="pp", name=f"pp{i}")
                          for i in range(HEC)]
                    for dt in range(KC):
                        xin = qin.tile([128, LCH], MDT, tag="qin")
                        nc.sync.dma_start(
                            xin[:],
                            in_ext[dt * 128:(dt + 1) * 128,
                                   lc * LCH:(lc + 1) * LCH])
                        for ec in range(HEC):
                            nc.tensor.matmul(
                                ps[ec][:],
                                wt[dt][:, ec * 128:(ec + 1) * 128],
                                xin[:],
                                start=(dt == 0), stop=(dt == KC - 1))
                    for ec in range(HEC):
                        dst = dest[ec][:, lc * LCH:(lc + 1) * LCH]
                        if which == "q":
                            # (psum + bq) * scale
                            nc.vector.tensor_scalar(
                                dst, ps[ec][:], float(SCALE),
                                bqs[:, ec:ec + 1], MULT, ADD)
                        else:
                            nc.vector.tensor_scalar(
                                dst, ps[ec][:], bkt[:, ec:ec + 1], None, ADD)

            # ---- phase 1b: v projection -> v_aug [s, NH*(E+1)] s-tiles
            wvt = []
            for dt in range(KC):
                w = wsl.tile([128, HE], MDT, tag="wsl", name="w_t")
                nc.sync.dma_start(w[:], wv_ext[dt * 128:(dt + 1) * 128, :])
                wvt.append(w)
            v_aug = []
            for st in range(NST):
                v = acts.tile([128, NH * (E + 1)], MDT, tag=f"vaug{st}",
                              name=f"vaug{st}")
                v_aug.append(v)
                nc.vector.tensor_copy(
                    v.rearrange("p (h u) -> p h u", u=E + 1)[:, :, E:E + 1],
                    ones_t.rearrange("p (h o) -> p h o", o=1))
                psv = pp.tile([128, HE], F32, tag="pp", name="psv")
                for dt in range(KC):
                    vin = qin.tile([128, 128], MDT, tag="vin")
                    nc.sync.dma_start(
                        vin[:],
                        vT_ext[dt * 128:(dt + 1) * 128,
                               st * 128:(st + 1) * 128])
                    nc.tensor.matmul(psv[:], vin[:], wvt[dt][:],
                                     start=(dt == 0), stop=(dt == KC - 1))
                nc.vector.tensor_add(
                    v.rearrange("p (h u) -> p h u", u=E + 1)[:, :, 0:E],
                    psv.rearrange("p (h e) -> p h e", e=E)[:],
                    bv_bc.rearrange("p (h e) -> p h e", e=E)[:])

            pp_ctx.__exit__(None, None, None)

            if taps:
                for i in range(HEC):
                    nc.sync.dma_start(dbg_qT[i * 128:(i + 1) * 128, :],
                                      qT_sb[i][:])
                    nc.sync.dma_start(dbg_kT[i * 128:(i + 1) * 128, :],
                                      kT_sb[i][:])
                for st in range(NST):
                    nc.sync.dma_start(dbg_v[st * 128:(st + 1) * 128, :],
                                      v_aug[st][:])

            # ---- phase 2+3: attention + output projection per head pair
            # l-chunks are processed G at a time sharing one wide scores
            # psum tile so the exp runs on G*LCH columns per instruction
            G = 2 if NLC % 2 == 0 else 1
            with (
                tc.tile_pool(name="psc", bufs=2, space="PSUM") as psc,
                tc.tile_pool(name="pav", bufs=2, space="PSUM") as pav,
                tc.tile_pool(name="pout", bufs=2, space="PSUM") as pout,
            ):
                for hp in range(NHP):
                    dups = []
                    for loc in range(2):  # head A (partitions 0:64), head B
                        p0 = loc * 64
                        h = 2 * hp + loc
                        dup = attnd.tile([128, L], MDT, tag="attnd",
                                         name="dup")
                        dups.append(dup)
                        for lcw in range(NLC // G):
                            lcs = [lcw * G + g for g in range(G)]
                            pavs = [pav.tile([65, LCH], F32, tag="pav",
                                             name="pavx") for _ in lcs]
                            for st in range(NST):
                                sc = psc.tile([128, G * LCH], F32,
                                              tag="psc", name="sc")
                                for g, lc in enumerate(lcs):
                                    nc.tensor.matmul(
                                        sc[:, g * LCH:(g + 1) * LCH],
                                        kT_sb[hp][p0:p0 + 64,
                                                  st * 128:(st + 1) * 128],
                                        qT_sb[hp][p0:p0 + 64,
                                                  lc * LCH:(lc + 1) * LCH],
                                        start=True, stop=True)
                                ex = expp.tile([128, G * LCH], MDT,
                                               tag="exp", name="ex")
                                nc.scalar.activation(ex[:], sc[:], EXP)
                                if taps and hp == 0 and loc == 0 \
                                        and lcw == 0 and st == 0:
                                    nc.sync.dma_start(dbg_ex[:],
                                                      ex[:, 0:LCH])
                                for g in range(G):
                                    nc.tensor.matmul(
                                        pavs[g][:],
                                        v_aug[st][:, h * (E + 1):
                                                  (h + 1) * (E + 1)],
                                        ex[:, g * LCH:(g + 1) * LCH],
                                        start=(st == 0),
                                        stop=(st == NST - 1))
                            for g, lc in enumerate(lcs):
                                rc = small.tile([1, LCH], F32, tag="rc",
                                                name="rc")
                                nc.vector.reciprocal(
                                    rc[:], pavs[g][64:65, :])
                                bc = small.tile([64, LCH], F32, tag="bc",
                                                name="bc")
                                nc.gpsimd.partition_broadcast(bc[:], rc[:],
                                                              channels=64)
                                nc.vector.tensor_mul(
                                    dup[0:64, lc * LCH:(lc + 1) * LCH],
                                    pavs[g][0:64, :], bc[:])
                        # shifted self-copy: partition 64+e, col l holds
                        # attn[e, l+1]; a stride-16 AP at offset 2t then
                        # reads the (2t, 2t+1) j-pair as one K=128
                        # stationary operand for the output projection
                        nc.sync.dma_start(dup[64:128, 0:L - 1],
                                          dup[0:64, 1:L])
                        if taps and hp == 0 and loc == 0:
                            nc.sync.dma_start(dbg_dup[:, 0:L - 1],
                                              dup[:, 0:L - 1])

                    for loc in range(2):
                        h = 2 * hp + loc
                        dup = dups[loc]
                        lhs = dup.rearrange("p (r j) -> p j r", j=J)
                        for dc in range(NDC):
                            po = pout.tile([R, DCH], F32, tag="pout",
                                           name="po")
                            for t in range(JE // 128):
                                nc.tensor.matmul(
                                    po[:],
                                    lhs[:, 2 * t, :],
                                    wo_sb[t][:, dc * DCH:(dc + 1) * DCH],
                                    start=(t == 0),
                                    stop=(t == JE // 128 - 1))
                            ob = outp.tile([R, DCH], F32, tag="outp",
                                           name="ob")
                            nc.vector.tensor_add(
                                ob[:], po[:],
                                bo_bc[0:R, dc * DCH:(dc + 1) * DCH])
                            nc.sync.dma_start(
                                out_ext[h * R:(h + 1) * R,
                                        dc * DCH:(dc + 1) * DCH],
                                ob[:])

    nc.compile()
    return nc


# ---------------------------------------------------------------------------
# host side
# ---------------------------------------------------------------------------

_NC_CACHE = {}

FULL_KEY = (2048, 1024, 8, 1024, "bf16")


def _get_nc(key=FULL_KEY):
    if key not in _NC_CACHE:
        _NC_CACHE[key] = build_core_kernel(*key)
    return _NC_CACHE[key]


def _np_mm_dtype(mm_dt):
    if mm_dt == "bf16":
        import ml_dtypes
        return ml_dtypes.bfloat16
    return np.float32


def make_in_maps(queries, keys, values, Wq, bq, Wk, bk, Wv, bv, Wo, bo,
                 mm_dt="f32r"):
    """Shard: core c handles batch c//2, heads NH*(c%2) .. NH*(c%2)+NH."""
    f = np.float32
    md = _np_mm_dtype(mm_dt)
    half_w = np.asarray(Wq).shape[1] // 2
    in_maps = []
    for c in range(8):
        b, half = c // 2, c % 2
        cs = slice(half * half_w, (half + 1) * half_w)
        in_maps.append({
            "qT": np.ascontiguousarray(np.asarray(queries[b], f).T.astype(md)),
            "kT": np.ascontiguousarray(np.asarray(keys[b], f).T.astype(md)),
            "vT": np.ascontiguousarray(np.asarray(values[b], f).T.astype(md)),
            "wq": np.ascontiguousarray(np.asarray(Wq, f)[:, cs].astype(md)),
            "wk": np.ascontiguousarray(np.asarray(Wk, f)[:, cs].astype(md)),
            "wv": np.ascontiguousarray(np.asarray(Wv, f)[:, cs].astype(md)),
            "bq": np.ascontiguousarray(np.asarray(bq, f)[cs]),
            "bk": np.ascontiguousarray(np.asarray(bk, f)[cs]),
            "bv": np.ascontiguousarray(np.asarray(bv, f)[cs]),
            "wo": np.ascontiguousarray(np.asarray(Wo, f).astype(md)),
            "bo": np.ascontiguousarray(np.asarray(bo, f)),
        })
    return in_maps


def assemble_output(results, B=4, L=2048, OUT_D=1024):
    out = np.empty((B, L, OUT_D), np.float32)
    half_rows = L // 2
    for c in range(8):
        b, half = c // 2, c % 2
        out[b, half * half_rows:(half + 1) * half_rows, :] = results[c]["out"]
    return out


def run_on_hw(inputs, trace=False, key=FULL_KEY, **kw):
    nc = _get_nc(key)
    in_maps = make_in_maps(**inputs, mm_dt=key[4])
    res = run_bass_kernel_spmd(nc, in_maps, core_ids=list(range(8)),
                               trace=trace, **kw)
    return assemble_output(res.results), res


def kernel(**inputs) -> np.ndarray:
    out, _ = run_on_hw(inputs, trace=False)
    return out


# revision 23
# speedup vs baseline: 1.3816x; 1.1207x over previous
"""Trainium2 Bass kernel for the nn_AttentionLayer problem.

Full multi-head attention layer, B=4, L=S=2048, d_model=1024, 16 heads of
dim 64, with the reference's "mix=True" transpose-then-flatten before the
output projection.

Key observation: the mix reshape means output row l' = h*128 + l//16 of each
batch depends ONLY on head h.  So sharding 8 cores as (batch, half-of-heads)
makes every core produce a disjoint, contiguous 1024-row slice of the output
with zero cross-core communication.

Per-core dataflow:
  - inputs arrive host-transposed ([D, L]) so the QKV projections contract
    d_model on the partition dim with natural weight layouts
  - q/k projections produce qT/kT [e, l] (heads on partitions, pairs of
    heads per 128-partition tile), with 1/sqrt(64) and bias folded in
  - scoresT tile [s=128, l=512] = kT.T @ qT per (s-tile, l-chunk); exp on
    the scalar engine; A@V accumulates v_aug = [v | ones] stationary so
    row 64 of the accumulator is the softmax denominator
  - normalize with vector-reciprocal + gpsimd partition_broadcast
  - the normalized attn [64, L] is self-copied (SBUF->SBUF DMA) to
    partitions 64..127 shifted by one position, which makes the output
    projection a clean K=128 matmul against natural Wo row-pair tiles

All matmuls run in `mm_dt` (float32r by default: full-rate fp32 on the PE).
"""

import numpy as np

import concourse.bass as bass
import concourse.mybir as mybir
import concourse.tile as tile
from concourse import bacc
from concourse.bass_utils import run_bass_kernel_spmd

F32 = mybir.dt.float32
EXP = mybir.ActivationFunctionType.Exp
MULT = mybir.AluOpType.mult
ADD = mybir.AluOpType.add

E = 64          # head dim
J = 16          # mix factor: total heads in the reference model
JE = J * E      # 1024 rows of Wo

MM_DTS = {
    "f32r": mybir.dt.float32r,
    "f32": mybir.dt.float32,
    "bf16": mybir.dt.bfloat16,
}


def build_core_kernel(L=2048, D=1024, NH=8, OUT_D=1024, mm_dt="f32r",
                      taps=False):
    """Builds the per-core Bacc graph (SPMD: all 8 cores run this)."""
    HE = NH * E               # projected width per core
    HEC = HE // 128           # qT/kT tiles (head pairs)
    NHP = NH // 2             # head pairs
    KC = D // 128             # contraction tiles for projections
    R = L // J                # output rows per head
    LCH = min(512, L)         # l-chunk
    NLC = L // LCH
    NST = L // 128            # s-tiles
    DCH = min(512, OUT_D)     # out-proj n-chunk
    NDC = OUT_D // DCH
    SCALE = 1.0 / np.sqrt(E)
    assert L % J == 0 and R <= 128 and HE % 128 == 0

    MDT = MM_DTS[mm_dt]

    nc = bacc.Bacc("TRN2", target_bir_lowering=False, debug=False,
                   enable_asserts=False)

    qT_ext = nc.declare_dram_parameter("qT", [D, L], MDT, isOutput=False)
    kT_ext = nc.declare_dram_parameter("kT", [D, L], MDT, isOutput=False)
    vT_ext = nc.declare_dram_parameter("vT", [D, L], MDT, isOutput=False)
    wq_ext = nc.declare_dram_parameter("wq", [D, HE], MDT, isOutput=False)
    wk_ext = nc.declare_dram_parameter("wk", [D, HE], MDT, isOutput=False)
    wv_ext = nc.declare_dram_parameter("wv", [D, HE], MDT, isOutput=False)
    bq_ext = nc.declare_dram_parameter("bq", [HE], F32, isOutput=False)
    bk_ext = nc.declare_dram_parameter("bk", [HE], F32, isOutput=False)
    bv_ext = nc.declare_dram_parameter("bv", [HE], F32, isOutput=False)
    wo_ext = nc.declare_dram_parameter("wo", [JE, OUT_D], MDT, isOutput=False)
    bo_ext = nc.declare_dram_parameter("bo", [OUT_D], F32, isOutput=False)
    out_ext = nc.declare_dram_parameter("out", [NH * R, OUT_D], F32,
                                        isOutput=True)
    HEC_ = HE // 128
    if taps:
        dbg_qT = nc.declare_dram_parameter("dbg_qT", [HEC_ * 128, L], MDT,
                                           isOutput=True)
        dbg_kT = nc.declare_dram_parameter("dbg_kT", [HEC_ * 128, L], MDT,
                                           isOutput=True)
        dbg_v = nc.declare_dram_parameter("dbg_v", [(L // 128) * 128,
                                                    NH * (E + 1)], MDT,
                                          isOutput=True)
        dbg_ex = nc.declare_dram_parameter("dbg_ex", [128, min(512, L)], MDT,
                                           isOutput=True)
        dbg_dup = nc.declare_dram_parameter("dbg_dup", [128, L], MDT,
                                            isOutput=True)

    with tile.TileContext(nc) as tc:
        with (
            tc.tile_pool(name="const", bufs=1) as const,
            tc.tile_pool(name="wsl", bufs=min(KC + 1, 3 * KC)) as wsl,
            tc.tile_pool(name="qin", bufs=3) as qin,
            tc.tile_pool(name="acts", bufs=1) as acts,
            tc.tile_pool(name="expp", bufs=6) as expp,
            tc.tile_pool(name="attnd", bufs=3) as attnd,
            tc.tile_pool(name="small", bufs=2) as small,
            tc.tile_pool(name="outp", bufs=4) as outp,
        ):
            # ---- constants ----
            bqt = const.tile([128, HEC], F32, tag="bqt")
            nc.sync.dma_start(bqt[:], bq_ext.rearrange("(c p) -> p c", p=128))
            bqs = const.tile([128, HEC], F32, tag="bqs")
            nc.vector.tensor_scalar_mul(bqs[:], bqt[:], float(SCALE))
            bkt = const.tile([128, HEC], F32, tag="bkt")
            nc.sync.dma_start(bkt[:], bk_ext.rearrange("(c p) -> p c", p=128))

            bv_row = const.tile([1, HE], F32, tag="bv_row")
            nc.sync.dma_start(bv_row[:],
                              bv_ext.rearrange("(o he) -> o he", o=1))
            bv_bc = const.tile([128, HE], F32, tag="bv_bc")
            nc.gpsimd.partition_broadcast(bv_bc[:], bv_row[:], channels=128)

            bo_row = const.tile([1, OUT_D], F32, tag="bo_row")
            nc.sync.dma_start(bo_row[:],
                              bo_ext.rearrange("(o d) -> o d", o=1))
            bo_bc = const.tile([128, OUT_D], F32, tag="bo_bc")
            nc.gpsimd.partition_broadcast(bo_bc[:], bo_row[:], channels=128)

            ones_t = const.tile([128, NH], F32, tag="ones_t")
            nc.vector.memset(ones_t[:], 1.0)

            wo_sb = []
            for t in range(JE // 128):
                w = const.tile([128, OUT_D], MDT, tag=f"wo{t}",
                               name=f"wo_sb{t}")
                nc.sync.dma_start(w[:], wo_ext[t * 128:(t + 1) * 128, :])
                wo_sb.append(w)

            # ---- phase 1: q/k projections -> qT/kT [e, l] head-pair tiles
            qT_sb = [acts.tile([128, L], MDT, tag=f"qT{i}", name=f"qT_sb{i}")
                     for i in range(HEC)]
            kT_sb = [acts.tile([128, L], MDT, tag=f"kT{i}", name=f"kT_sb{i}")
                     for i in range(HEC)]

            pp_ctx = tc.tile_pool(name="pp", bufs=min(HEC + 2, 6),
                                  space="PSUM")
            pp = pp_ctx.__enter__()

            for which, w_ext, in_ext, dest in (
                ("q", wq_ext, qT_ext, qT_sb),
                ("k", wk_ext, kT_ext, kT_sb),
            ):
                wt = []
                for dt in range(KC):
                    w = wsl.tile([128, HE], MDT, tag="wsl", name="w_t")
                    nc.sync.dma_start(w[:], w_ext[dt * 128:(dt + 1) * 128, :])
                    wt.append(w)
                for lc in range(NLC):
                    ps = [pp.tile([128, LCH], F32, tag="pp", name=f"pp{i}")
                          for i in range(HEC)]
                    for dt in range(KC):
                        xin = qin.tile([128, LCH], MDT, tag="qin")
                        nc.sync.dma_start(
                            xin[:],
                            in_ext[dt * 128:(dt + 1) * 128,
                                   lc * LCH:(lc + 1) * LCH])
                        for ec in range(HEC):
                            nc.tensor.matmul(
                                ps[ec][:],
                                wt[dt][:, ec * 128:(ec + 1) * 128],
                                xin[:],
                                start=(dt == 0), stop=(dt == KC - 1))
                    for ec in range(HEC):
                        dst = dest[ec][:, lc * LCH:(lc + 1) * LCH]
                        if which == "q":
                            # (psum + bq) * scale
                            nc.vector.tensor_scalar(
                                dst, ps[ec][:], float(SCALE),
                                bqs[:, ec:ec + 1], MULT, ADD)
                        else:
                            nc.vector.tensor_scalar(
                                dst, ps[ec][:], bkt[:, ec:ec + 1], None, ADD)

            # ---- phase 1b: v projection -> v_aug [s, NH*(E+1)] s-tiles
            wvt = []
            for dt in range(KC):
                w = wsl.tile([128, HE], MDT, tag="wsl", name="w_t")
                nc.sync.dma_start(w[:], wv_ext[dt * 128:(dt + 1) * 128, :])
                wvt.append(w)
            v_aug = []
            for st in range(NST):
                v = acts.tile([128, NH * (E + 1)], MDT, tag=f"vaug{st}",
                              name=f"vaug{st}")
                v_aug.append(v)
                nc.vector.tensor_copy(
                    v.rearrange("p (h u) -> p h u", u=E + 1)[:, :, E:E + 1],
                    ones_t.rearrange("p (h o) -> p h o", o=1))
                psv = pp.tile([128, HE], F32, tag="pp", name="psv")
                for dt in range(KC):
                    vin = qin.tile([128, 128], MDT, tag="vin")
                    nc.sync.dma_start(
                        vin[:],
                        vT_ext[dt * 128:(dt + 1) * 128,
                               st * 128:(st + 1) * 128])
                    nc.tensor.matmul(psv[:], vin[:], wvt[dt][:],
                                     start=(dt == 0), stop=(dt == KC - 1))
                nc.vector.tensor_add(
                    v.rearrange("p (h u) -> p h u", u=E + 1)[:, :, 0:E],
                    psv.rearrange("p (h e) -> p h e", e=E)[:],
                    bv_bc.rearrange("p (h e) -> p h e", e=E)[:])

            pp_ctx.__exit__(None, None, None)

            if taps:
                for i in range(HEC):
                    nc.sync.dma_start(dbg_qT[i * 128:(i + 1) * 128, :],
                                      qT_sb[i][:])
                    nc.sync.dma_start(dbg_kT[i * 128:(i + 1) * 128, :],
                                      kT_sb[i][:])
                for st in range(NST):
                    nc.sync.dma_start(dbg_v[st * 128:(st + 1) * 128, :],
                                      v_aug[st][:])

            # ---- phase 2+3: attention + output projection per head pair
            # l-chunks are processed G at a time sharing one wide scores
            # psum tile so the exp runs on G*LCH columns per instruction
            G = 2 if NLC % 2 == 0 else 1
            with (
                tc.tile_pool(name="psc", bufs=2, space="PSUM") as psc,
                tc.tile_pool(name="pav", bufs=2, space="PSUM") as pav,
                tc.tile_pool(name="pout", bufs=2, space="PSUM") as pout,
            ):
                for hp in range(NHP):
                    dups = []
                    for loc in range(2):  # head A (partitions 0:64), head B
                        p0 = loc * 64
                        h = 2 * hp + loc
                        dup = attnd.tile([128, L], MDT, tag="attnd",
                                         name="dup")
                        dups.append(dup)
                        for lcw in range(NLC // G):
                            lcs = [lcw * G + g for g in range(G)]
                            pavs = [pav.tile([65, LCH], F32, tag="pav",
                                             name="pavx") for _ in lcs]
                            for st in range(NST):
                                sc = psc.tile([128, G * LCH], F32,
                                              tag="psc", name="sc")
                                for g, lc in enumerate(lcs):
                                    nc.tensor.matmul(
                                        sc[:, g * LCH:(g + 1) * LCH],
                                        kT_sb[hp][p0:p0 + 64,
                                                  st * 128:(st + 1) * 128],
                                        qT_sb[hp][p0:p0 + 64,
                                                  lc * LCH:(lc + 1) * LCH],
                                        start=True, stop=True)
                                ex = expp.tile([128, G * LCH], MDT,
                                               tag="exp", name="ex")
                                nc.scalar.activation(ex[:], sc[:], EXP)
                                if taps and hp == 0 and loc == 0 \
                                        and lcw == 0 and st == 0:
                                    nc.sync.dma_start(dbg_ex[:],
                                                      ex[:, 0:LCH])
                                for g in range(G):
                                    nc.tensor.matmul(
                                        pavs[g][:],
                                        v_aug[st][:, h * (E + 1):
                                                  (h + 1) * (E + 1)],
                                        ex[:, g * LCH:(g + 1) * LCH],
                                        start=(st == 0),
                                        stop=(st == NST - 1))
                            for g, lc in enumerate(lcs):
                                # quick-release copy: frees the PSUM bank so
                                # the next A@V round isn't stalled behind the
                                # (slow) reciprocal
                                pcp = small.tile([65, LCH], F32, tag="pcp",
                                                 name="pcp", bufs=4)
                                nc.vector.tensor_copy(pcp[:], pavs[g][:])
                                rc = small.tile([1, LCH], F32, tag="rc",
                                                name="rc")
                                nc.vector.reciprocal(
                                    rc[:], pcp[64:65, :])
                                bc = small.tile([64, LCH], F32, tag="bc",
                                                name="bc")
                                nc.gpsimd.partition_broadcast(bc[:], rc[:],
                                                              channels=64)
                                nc.vector.tensor_mul(
                                    dup[0:64, lc * LCH:(lc + 1) * LCH],
                                    pcp[0:64, :], bc[:])
                        # shifted self-copy: partition 64+e, col l holds
                        # attn[e, l+1]; a stride-16 AP at offset 2t then
                        # reads the (2t, 2t+1) j-pair as one K=128
                        # stationary operand for the output projection
                        nc.sync.dma_start(dup[64:128, 0:L - 1],
                                          dup[0:64, 1:L])
                        if taps and hp == 0 and loc == 0:
                            nc.sync.dma_start(dbg_dup[:, 0:L - 1],
                                              dup[:, 0:L - 1])

                    for loc in range(2):
                        h = 2 * hp + loc
                        dup = dups[loc]
                        lhs = dup.rearrange("p (r j) -> p j r", j=J)
                        for dc in range(NDC):
                            po = pout.tile([R, DCH], F32, tag="pout",
                                           name="po")
                            for t in range(JE // 128):
                                nc.tensor.matmul(
                                    po[:],
                                    lhs[:, 2 * t, :],
                                    wo_sb[t][:, dc * DCH:(dc + 1) * DCH],
                                    start=(t == 0),
                                    stop=(t == JE // 128 - 1))
                            ob = outp.tile([R, DCH], F32, tag="outp",
                                           name="ob")
                            nc.vector.tensor_add(
                                ob[:], po[:],
                                bo_bc[0:R, dc * DCH:(dc + 1) * DCH])
                            nc.sync.dma_start(
                                out_ext[h * R:(h + 1) * R,
                                        dc * DCH:(dc + 1) * DCH],
                                ob[:])

    nc.compile()
    return nc


# ---------------------------------------------------------------------------
# host side
# ---------------------------------------------------------------------------

_NC_CACHE = {}

FULL_KEY = (2048, 1024, 8, 1024, "bf16")


def _get_nc(key=FULL_KEY):
    if key not in _NC_CACHE:
        _NC_CACHE[key] = build_core_kernel(*key)
    return _NC_CACHE[key]


def _np_mm_dtype(mm_dt):
    if mm_dt == "bf16":
        import ml_dtypes
        return ml_dtypes.bfloat16
    return np.float32


def make_in_maps(queries, keys, values, Wq, bq, Wk, bk, Wv, bv, Wo, bo,
                 mm_dt="f32r"):
    """Shard: core c handles batch c//2, heads NH*(c%2) .. NH*(c%2)+NH."""
    f = np.float32
    md = _np_mm_dtype(mm_dt)
    half_w = np.asarray(Wq).shape[1] // 2
    in_maps = []
    for c in range(8):
        b, half = c // 2, c % 2
        cs = slice(half * half_w, (half + 1) * half_w)
        in_maps.append({
            "qT": np.ascontiguousarray(np.asarray(queries[b], f).T.astype(md)),
            "kT": np.ascontiguousarray(np.asarray(keys[b], f).T.astype(md)),
            "vT": np.ascontiguousarray(np.asarray(values[b], f).T.astype(md)),
            "wq": np.ascontiguousarray(np.asarray(Wq, f)[:, cs].astype(md)),
            "wk": np.ascontiguousarray(np.asarray(Wk, f)[:, cs].astype(md)),
            "wv": np.ascontiguousarray(np.asarray(Wv, f)[:, cs].astype(md)),
            "bq": np.ascontiguousarray(np.asarray(bq, f)[cs]),
            "bk": np.ascontiguousarray(np.asarray(bk, f)[cs]),
            "bv": np.ascontiguousarray(np.asarray(bv, f)[cs]),
            "wo": np.ascontiguousarray(np.asarray(Wo, f).astype(md)),
            "bo": np.ascontiguousarray(np.asarray(bo, f)),
        })
    return in_maps


def assemble_output(results, B=4, L=2048, OUT_D=1024):
    out = np.empty((B, L, OUT_D), np.float32)
    half_rows = L // 2
    for c in range(8):
        b, half = c // 2, c % 2
        out[b, half * half_rows:(half + 1) * half_rows, :] = results[c]["out"]
    return out


def run_on_hw(inputs, trace=False, key=FULL_KEY, **kw):
    nc = _get_nc(key)
    in_maps = make_in_maps(**inputs, mm_dt=key[4])
    res = run_bass_kernel_spmd(nc, in_maps, core_ids=list(range(8)),
                               trace=trace, **kw)
    return assemble_output(res.results), res


def kernel(**inputs) -> np.ndarray:
    out, _ = run_on_hw(inputs, trace=False)
    return out


# revision 24
# speedup vs baseline: 1.6352x; 1.1836x over previous
"""Trainium2 Bass kernel for the nn_AttentionLayer problem.

Full multi-head attention layer, B=4, L=S=2048, d_model=1024, 16 heads of
dim 64, with the reference's "mix=True" transpose-then-flatten before the
output projection.

Key observation: the mix reshape means output row l' = h*128 + l//16 of each
batch depends ONLY on head h.  So sharding 8 cores as (batch, half-of-heads)
makes every core produce a disjoint, contiguous 1024-row slice of the output
with zero cross-core communication.

Per-core dataflow:
  - inputs arrive host-transposed ([D, L]) so the QKV projections contract
    d_model on the partition dim with natural weight layouts
  - q/k projections produce qT/kT [e, l] (heads on partitions, pairs of
    heads per 128-partition tile), with 1/sqrt(64) and bias folded in
  - scoresT tile [s=128, l=512] = kT.T @ qT per (s-tile, l-chunk); exp on
    the scalar engine; A@V accumulates v_aug = [v | ones] stationary so
    row 64 of the accumulator is the softmax denominator
  - normalize with vector-reciprocal + gpsimd partition_broadcast
  - the normalized attn [64, L] is self-copied (SBUF->SBUF DMA) to
    partitions 64..127 shifted by one position, which makes the output
    projection a clean K=128 matmul against natural Wo row-pair tiles

All matmuls run in `mm_dt` (float32r by default: full-rate fp32 on the PE).
"""

import numpy as np

import concourse.bass as bass
import concourse.mybir as mybir
import concourse.tile as tile
from concourse import bacc
from concourse.bass_utils import run_bass_kernel_spmd

F32 = mybir.dt.float32
EXP = mybir.ActivationFunctionType.Exp
MULT = mybir.AluOpType.mult
ADD = mybir.AluOpType.add

E = 64          # head dim
J = 16          # mix factor: total heads in the reference model
JE = J * E      # 1024 rows of Wo

MM_DTS = {
    "f32r": mybir.dt.float32r,
    "f32": mybir.dt.float32,
    "bf16": mybir.dt.bfloat16,
}


def build_core_kernel(L=2048, D=1024, NH=8, OUT_D=1024, mm_dt="f32r",
                      taps=False):
    """Builds the per-core Bacc graph (SPMD: all 8 cores run this)."""
    HE = NH * E               # projected width per core
    HEC = HE // 128           # qT/kT tiles (head pairs)
    NHP = NH // 2             # head pairs
    KC = D // 128             # contraction tiles for projections
    R = L // J                # output rows per head
    LCH = min(512, L)         # l-chunk
    NLC = L // LCH
    NST = L // 128            # s-tiles
    DCH = min(512, OUT_D)     # out-proj n-chunk
    NDC = OUT_D // DCH
    SCALE = 1.0 / np.sqrt(E)
    assert L % J == 0 and R <= 128 and HE % 128 == 0

    MDT = MM_DTS[mm_dt]

    nc = bacc.Bacc("TRN2", target_bir_lowering=False, debug=False,
                   enable_asserts=False)

    qT_ext = nc.declare_dram_parameter("qT", [D, L], MDT, isOutput=False)
    kT_ext = nc.declare_dram_parameter("kT", [D, L], MDT, isOutput=False)
    vT_ext = nc.declare_dram_parameter("vT", [D, L], MDT, isOutput=False)
    wq_ext = nc.declare_dram_parameter("wq", [D, HE], MDT, isOutput=False)
    wk_ext = nc.declare_dram_parameter("wk", [D, HE], MDT, isOutput=False)
    wv_ext = nc.declare_dram_parameter("wv", [D, HE], MDT, isOutput=False)
    bq_ext = nc.declare_dram_parameter("bq", [HE], F32, isOutput=False)
    bk_ext = nc.declare_dram_parameter("bk", [HE], F32, isOutput=False)
    bv_ext = nc.declare_dram_parameter("bv", [HE], F32, isOutput=False)
    wo_ext = nc.declare_dram_parameter("wo", [JE, OUT_D], MDT, isOutput=False)
    bo_ext = nc.declare_dram_parameter("bo", [OUT_D], F32, isOutput=False)
    out_ext = nc.declare_dram_parameter("out", [NH * R, OUT_D], F32,
                                        isOutput=True)
    HEC_ = HE // 128
    if taps:
        dbg_qT = nc.declare_dram_parameter("dbg_qT", [HEC_ * 128, L], MDT,
                                           isOutput=True)
        dbg_kT = nc.declare_dram_parameter("dbg_kT", [HEC_ * 128, L], MDT,
                                           isOutput=True)
        dbg_v = nc.declare_dram_parameter("dbg_v", [(L // 128) * 128,
                                                    NH * (E + 1)], MDT,
                                          isOutput=True)
        dbg_ex = nc.declare_dram_parameter("dbg_ex", [128, min(512, L)], MDT,
                                           isOutput=True)
        dbg_dup = nc.declare_dram_parameter("dbg_dup", [128, L], MDT,
                                            isOutput=True)

    with tile.TileContext(nc) as tc:
        with (
            tc.tile_pool(name="const", bufs=1) as const,
            tc.tile_pool(name="wsl", bufs=min(KC + 1, 3 * KC)) as wsl,
            tc.tile_pool(name="qin", bufs=3) as qin,
            tc.tile_pool(name="acts", bufs=1) as acts,
            tc.tile_pool(name="expp", bufs=6) as expp,
            tc.tile_pool(name="attnd", bufs=3) as attnd,
            tc.tile_pool(name="small", bufs=2) as small,
            tc.tile_pool(name="outp", bufs=4) as outp,
        ):
            # ---- constants ----
            bqt = const.tile([128, HEC], F32, tag="bqt")
            nc.sync.dma_start(bqt[:], bq_ext.rearrange("(c p) -> p c", p=128))
            bqs = const.tile([128, HEC], F32, tag="bqs")
            nc.vector.tensor_scalar_mul(bqs[:], bqt[:], float(SCALE))
            bkt = const.tile([128, HEC], F32, tag="bkt")
            nc.sync.dma_start(bkt[:], bk_ext.rearrange("(c p) -> p c", p=128))

            bv_row = const.tile([1, HE], F32, tag="bv_row")
            nc.sync.dma_start(bv_row[:],
                              bv_ext.rearrange("(o he) -> o he", o=1))
            bv_bc = const.tile([128, HE], F32, tag="bv_bc")
            nc.gpsimd.partition_broadcast(bv_bc[:], bv_row[:], channels=128)

            bo_row = const.tile([1, OUT_D], F32, tag="bo_row")
            nc.sync.dma_start(bo_row[:],
                              bo_ext.rearrange("(o d) -> o d", o=1))
            bo_bc = const.tile([128, OUT_D], F32, tag="bo_bc")
            nc.gpsimd.partition_broadcast(bo_bc[:], bo_row[:], channels=128)

            ones_t = const.tile([128, NH], F32, tag="ones_t")
            nc.vector.memset(ones_t[:], 1.0)

            wo_sb = []
            for t in range(JE // 128):
                w = const.tile([128, OUT_D], MDT, tag=f"wo{t}",
                               name=f"wo_sb{t}")
                nc.sync.dma_start(w[:], wo_ext[t * 128:(t + 1) * 128, :])
                wo_sb.append(w)

            # ---- phase 1: q/k projections -> qT/kT [e, l] head-pair tiles
            qT_sb = [acts.tile([128, L], MDT, tag=f"qT{i}", name=f"qT_sb{i}")
                     for i in range(HEC)]
            kT_sb = [acts.tile([128, L], MDT, tag=f"kT{i}", name=f"kT_sb{i}")
                     for i in range(HEC)]

            pp_ctx = tc.tile_pool(name="pp", bufs=min(NLC + 2, 6),
                                  space="PSUM")
            pp = pp_ctx.__enter__()

            def load_w(w_ext):
                wt = []
                for dt in range(KC):
                    w = wsl.tile([128, HE], MDT, tag="wsl", name="w_t")
                    nc.sync.dma_start(w[:],
                                      w_ext[dt * 128:(dt + 1) * 128, :])
                    wt.append(w)
                return wt

            def load_xin(in_ext):
                # full-row d-tiles: one big DMA each, live for the phase
                tiles = []
                for dt in range(KC):
                    x = qin.tile([128, L], MDT, tag=f"xin{dt}",
                                 name=f"xin{dt}", bufs=1)
                    nc.sync.dma_start(x[:],
                                      in_ext[dt * 128:(dt + 1) * 128, :])
                    tiles.append(x)
                return tiles

            # ---- v projection first (A@V consumes every s-tile of v)
            wvt = load_w(wv_ext)
            vin = load_xin(vT_ext)
            v_aug = []
            for st in range(NST):
                v = acts.tile([128, NH * (E + 1)], MDT, tag=f"vaug{st}",
                              name=f"vaug{st}")
                v_aug.append(v)
                nc.vector.tensor_copy(
                    v.rearrange("p (h u) -> p h u", u=E + 1)[:, :, E:E + 1],
                    ones_t.rearrange("p (h o) -> p h o", o=1))
            # two interleaved accumulation chains hide the PSUM RAW latency
            for sp in range(NST // 2):
                ps2 = [pp.tile([128, HE], F32, tag="pp", name=f"psv{i}")
                       for i in range(2)]
                for dt in range(KC):
                    for i in range(2):
                        st = 2 * sp + i
                        nc.tensor.matmul(
                            ps2[i][:],
                            vin[dt][:, st * 128:(st + 1) * 128],
                            wvt[dt][:],
                            start=(dt == 0), stop=(dt == KC - 1))
                for i in range(2):
                    st = 2 * sp + i
                    v = v_aug[st]
                    nc.vector.tensor_add(
                        v.rearrange("p (h u) -> p h u", u=E + 1)[:, :, 0:E],
                        ps2[i].rearrange("p (h e) -> p h e", e=E)[:],
                        bv_bc.rearrange("p (h e) -> p h e", e=E)[:])

            # ---- k then q projections; stationary weight slice serves all
            # NLC moving chunks (amortizes LDWEIGHTS 4x)
            for which, w_ext, in_ext, dest in (
                ("k", wk_ext, kT_ext, kT_sb),
                ("q", wq_ext, qT_ext, qT_sb),
            ):
                wt = load_w(w_ext)
                xin = load_xin(in_ext)
                for ec in range(HEC):
                    ps = [pp.tile([128, LCH], F32, tag="pp", name=f"pp{i}")
                          for i in range(NLC)]
                    for dt in range(KC):
                        for lc in range(NLC):
                            nc.tensor.matmul(
                                ps[lc][:],
                                wt[dt][:, ec * 128:(ec + 1) * 128],
                                xin[dt][:, lc * LCH:(lc + 1) * LCH],
                                start=(dt == 0), stop=(dt == KC - 1))
                    for lc in range(NLC):
                        dst = dest[ec][:, lc * LCH:(lc + 1) * LCH]
                        if which == "q":
                            # (psum + bq) * scale
                            nc.vector.tensor_scalar(
                                dst, ps[lc][:], float(SCALE),
                                bqs[:, ec:ec + 1], MULT, ADD)
                        else:
                            nc.vector.tensor_scalar(
                                dst, ps[lc][:], bkt[:, ec:ec + 1], None, ADD)

            pp_ctx.__exit__(None, None, None)

            if taps:
                for i in range(HEC):
                    nc.sync.dma_start(dbg_qT[i * 128:(i + 1) * 128, :],
                                      qT_sb[i][:])
                    nc.sync.dma_start(dbg_kT[i * 128:(i + 1) * 128, :],
                                      kT_sb[i][:])
                for st in range(NST):
                    nc.sync.dma_start(dbg_v[st * 128:(st + 1) * 128, :],
                                      v_aug[st][:])

            # ---- phase 2+3: attention + output projection per head pair
            # l-chunks are processed G at a time sharing one wide scores
            # psum tile so the exp runs on G*LCH columns per instruction
            G = 2 if NLC % 2 == 0 else 1
            with (
                tc.tile_pool(name="psc", bufs=2, space="PSUM") as psc,
                tc.tile_pool(name="pav", bufs=2, space="PSUM") as pav,
                tc.tile_pool(name="pout", bufs=2, space="PSUM") as pout,
            ):
                for hp in range(NHP):
                    dups = []
                    for loc in range(2):  # head A (partitions 0:64), head B
                        p0 = loc * 64
                        h = 2 * hp + loc
                        dup = attnd.tile([128, L], MDT, tag="attnd",
                                         name="dup")
                        dups.append(dup)
                        for lcw in range(NLC // G):
                            lcs = [lcw * G + g for g in range(G)]
                            pavs = [pav.tile([65, LCH], F32, tag="pav",
                                             name="pavx") for _ in lcs]
                            for st in range(NST):
                                sc = psc.tile([128, G * LCH], F32,
                                              tag="psc", name="sc")
                                for g, lc in enumerate(lcs):
                                    nc.tensor.matmul(
                                        sc[:, g * LCH:(g + 1) * LCH],
                                        kT_sb[hp][p0:p0 + 64,
                                                  st * 128:(st + 1) * 128],
                                        qT_sb[hp][p0:p0 + 64,
                                                  lc * LCH:(lc + 1) * LCH],
                                        start=True, stop=True)
                                ex = expp.tile([128, G * LCH], MDT,
                                               tag="exp", name="ex")
                                nc.scalar.activation(ex[:], sc[:], EXP)
                                if taps and hp == 0 and loc == 0 \
                                        and lcw == 0 and st == 0:
                                    nc.sync.dma_start(dbg_ex[:],
                                                      ex[:, 0:LCH])
                                for g in range(G):
                                    nc.tensor.matmul(
                                        pavs[g][:],
                                        v_aug[st][:, h * (E + 1):
                                                  (h + 1) * (E + 1)],
                                        ex[:, g * LCH:(g + 1) * LCH],
                                        start=(st == 0),
                                        stop=(st == NST - 1))
                            for g, lc in enumerate(lcs):
                                # quick-release copy: frees the PSUM bank so
                                # the next A@V round isn't stalled behind the
                                # (slow) reciprocal
                                pcp = small.tile([65, LCH], F32, tag="pcp",
                                                 name="pcp", bufs=4)
                                nc.vector.tensor_copy(pcp[:], pavs[g][:])
                                rc = small.tile([1, LCH], F32, tag="rc",
                                                name="rc")
                                nc.vector.reciprocal(
                                    rc[:], pcp[64:65, :])
                                bc = small.tile([64, LCH], F32, tag="bc",
                                                name="bc")
                                nc.gpsimd.partition_broadcast(bc[:], rc[:],
                                                              channels=64)
                                nc.vector.tensor_mul(
                                    dup[0:64, lc * LCH:(lc + 1) * LCH],
                                    pcp[0:64, :], bc[:])
                        # shifted self-copy: partition 64+e, col l holds
                        # attn[e, l+1]; a stride-16 AP at offset 2t then
                        # reads the (2t, 2t+1) j-pair as one K=128
                        # stationary operand for the output projection
                        nc.sync.dma_start(dup[64:128, 0:L - 1],
                                          dup[0:64, 1:L])
                        if taps and hp == 0 and loc == 0:
                            nc.sync.dma_start(dbg_dup[:, 0:L - 1],
                                              dup[:, 0:L - 1])

                    for loc in range(2):
                        h = 2 * hp + loc
                        dup = dups[loc]
                        lhs = dup.rearrange("p (r j) -> p j r", j=J)
                        for dc in range(NDC):
                            po = pout.tile([R, DCH], F32, tag="pout",
                                           name="po")
                            for t in range(JE // 128):
                                nc.tensor.matmul(
                                    po[:],
                                    lhs[:, 2 * t, :],
                                    wo_sb[t][:, dc * DCH:(dc + 1) * DCH],
                                    start=(t == 0),
                                    stop=(t == JE // 128 - 1))
                            ob = outp.tile([R, DCH], F32, tag="outp",
                                           name="ob")
                            nc.vector.tensor_add(
                                ob[:], po[:],
                                bo_bc[0:R, dc * DCH:(dc + 1) * DCH])
                            nc.sync.dma_start(
                                out_ext[h * R:(h + 1) * R,
                                        dc * DCH:(dc + 1) * DCH],
                                ob[:])

    nc.compile()
    return nc


# ---------------------------------------------------------------------------
# host side
# ---------------------------------------------------------------------------

_NC_CACHE = {}

FULL_KEY = (2048, 1024, 8, 1024, "bf16")


def _get_nc(key=FULL_KEY):
    if key not in _NC_CACHE:
        _NC_CACHE[key] = build_core_kernel(*key)
    return _NC_CACHE[key]


def _np_mm_dtype(mm_dt):
    if mm_dt == "bf16":
        import ml_dtypes
        return ml_dtypes.bfloat16
    return np.float32


def make_in_maps(queries, keys, values, Wq, bq, Wk, bk, Wv, bv, Wo, bo,
                 mm_dt="f32r"):
    """Shard: core c handles batch c//2, heads NH*(c%2) .. NH*(c%2)+NH."""
    f = np.float32
    md = _np_mm_dtype(mm_dt)
    half_w = np.asarray(Wq).shape[1] // 2
    in_maps = []
    for c in range(8):
        b, half = c // 2, c % 2
        cs = slice(half * half_w, (half + 1) * half_w)
        in_maps.append({
            "qT": np.ascontiguousarray(np.asarray(queries[b], f).T.astype(md)),
            "kT": np.ascontiguousarray(np.asarray(keys[b], f).T.astype(md)),
            "vT": np.ascontiguousarray(np.asarray(values[b], f).T.astype(md)),
            "wq": np.ascontiguousarray(np.asarray(Wq, f)[:, cs].astype(md)),
            "wk": np.ascontiguousarray(np.asarray(Wk, f)[:, cs].astype(md)),
            "wv": np.ascontiguousarray(np.asarray(Wv, f)[:, cs].astype(md)),
            "bq": np.ascontiguousarray(np.asarray(bq, f)[cs]),
            "bk": np.ascontiguousarray(np.asarray(bk, f)[cs]),
            "bv": np.ascontiguousarray(np.asarray(bv, f)[cs]),
            "wo": np.ascontiguousarray(np.asarray(Wo, f).astype(md)),
            "bo": np.ascontiguousarray(np.asarray(bo, f)),
        })
    return in_maps


def assemble_output(results, B=4, L=2048, OUT_D=1024):
    out = np.empty((B, L, OUT_D), np.float32)
    half_rows = L // 2
    for c in range(8):
        b, half = c // 2, c % 2
        out[b, half * half_rows:(half + 1) * half_rows, :] = results[c]["out"]
    return out


def run_on_hw(inputs, trace=False, key=FULL_KEY, **kw):
    nc = _get_nc(key)
    in_maps = make_in_maps(**inputs, mm_dt=key[4])
    res = run_bass_kernel_spmd(nc, in_maps, core_ids=list(range(8)),
                               trace=trace, **kw)
    return assemble_output(res.results), res


def kernel(**inputs) -> np.ndarray:
    out, _ = run_on_hw(inputs, trace=False)
    return out


# revision 28
# speedup vs baseline: 1.6514x; 1.0099x over previous
"""Trainium2 Bass kernel for the nn_AttentionLayer problem.

Full multi-head attention layer, B=4, L=S=2048, d_model=1024, 16 heads of
dim 64, with the reference's "mix=True" transpose-then-flatten before the
output projection.

Key observation: the mix reshape means output row l' = h*128 + l//16 of each
batch depends ONLY on head h.  So sharding 8 cores as (batch, half-of-heads)
makes every core produce a disjoint, contiguous 1024-row slice of the output
with zero cross-core communication.

Per-core dataflow:
  - inputs arrive host-transposed ([D, L]) so the QKV projections contract
    d_model on the partition dim with natural weight layouts
  - q/k projections produce qT/kT [e, l] (heads on partitions, pairs of
    heads per 128-partition tile), with 1/sqrt(64) and bias folded in
  - scoresT tile [s=128, l=512] = kT.T @ qT per (s-tile, l-chunk); exp on
    the scalar engine; A@V accumulates v_aug = [v | ones] stationary so
    row 64 of the accumulator is the softmax denominator
  - normalize with vector-reciprocal + gpsimd partition_broadcast
  - the normalized attn [64, L] is self-copied (SBUF->SBUF DMA) to
    partitions 64..127 shifted by one position, which makes the output
    projection a clean K=128 matmul against natural Wo row-pair tiles

All matmuls run in `mm_dt` (float32r by default: full-rate fp32 on the PE).
"""

import numpy as np

import concourse.bass as bass
import concourse.mybir as mybir
import concourse.tile as tile
from concourse import bacc
from concourse.bass_utils import run_bass_kernel_spmd

F32 = mybir.dt.float32
EXP = mybir.ActivationFunctionType.Exp
MULT = mybir.AluOpType.mult
ADD = mybir.AluOpType.add

E = 64          # head dim
J = 16          # mix factor: total heads in the reference model
JE = J * E      # 1024 rows of Wo

MM_DTS = {
    "f32r": mybir.dt.float32r,
    "f32": mybir.dt.float32,
    "bf16": mybir.dt.bfloat16,
}


def build_core_kernel(L=2048, D=1024, NH=8, OUT_D=1024, mm_dt="f32r",
                      taps=False):
    """Builds the per-core Bacc graph (SPMD: all 8 cores run this)."""
    HE = NH * E               # projected width per core
    HEC = HE // 128           # qT/kT tiles (head pairs)
    NHP = NH // 2             # head pairs
    KC = D // 128             # contraction tiles for projections
    R = L // J                # output rows per head
    LCH = min(512, L)         # l-chunk
    NLC = L // LCH
    NST = L // 128            # s-tiles
    DCH = min(512, OUT_D)     # out-proj n-chunk
    NDC = OUT_D // DCH
    SCALE = 1.0 / np.sqrt(E)
    assert L % J == 0 and R <= 128 and HE % 128 == 0

    MDT = MM_DTS[mm_dt]

    nc = bacc.Bacc("TRN2", target_bir_lowering=False, debug=False,
                   enable_asserts=False)

    qT_ext = nc.declare_dram_parameter("qT", [D, L], MDT, isOutput=False)
    kT_ext = nc.declare_dram_parameter("kT", [D, L], MDT, isOutput=False)
    vT_ext = nc.declare_dram_parameter("vT", [D, L], MDT, isOutput=False)
    wq_ext = nc.declare_dram_parameter("wq", [D, HE], MDT, isOutput=False)
    wk_ext = nc.declare_dram_parameter("wk", [D, HE], MDT, isOutput=False)
    wv_ext = nc.declare_dram_parameter("wv", [D, HE], MDT, isOutput=False)
    bq_ext = nc.declare_dram_parameter("bq", [HE], F32, isOutput=False)
    bk_ext = nc.declare_dram_parameter("bk", [HE], F32, isOutput=False)
    bv_ext = nc.declare_dram_parameter("bv", [HE], F32, isOutput=False)
    wo_ext = nc.declare_dram_parameter("wo", [JE, OUT_D], MDT, isOutput=False)
    bo_ext = nc.declare_dram_parameter("bo", [OUT_D], F32, isOutput=False)
    out_ext = nc.declare_dram_parameter("out", [NH * R, OUT_D], F32,
                                        isOutput=True)
    HEC_ = HE // 128
    if taps:
        dbg_qT = nc.declare_dram_parameter("dbg_qT", [HEC_ * 128, L], MDT,
                                           isOutput=True)
        dbg_kT = nc.declare_dram_parameter("dbg_kT", [HEC_ * 128, L], MDT,
                                           isOutput=True)
        dbg_v = nc.declare_dram_parameter("dbg_v", [(L // 128) * 128,
                                                    NH * (E + 1)], MDT,
                                          isOutput=True)
        dbg_ex = nc.declare_dram_parameter("dbg_ex", [128, min(512, L)], MDT,
                                           isOutput=True)
        dbg_dup = nc.declare_dram_parameter("dbg_dup", [128, L], MDT,
                                            isOutput=True)

    with tile.TileContext(nc) as tc:
        with (
            tc.tile_pool(name="const", bufs=1) as const,
            tc.tile_pool(name="wsl", bufs=min(KC + 1, 3 * KC)) as wsl,
            tc.tile_pool(name="qin", bufs=3) as qin,
            tc.tile_pool(name="acts", bufs=1) as acts,
            tc.tile_pool(name="expp", bufs=6) as expp,
            tc.tile_pool(name="attnd", bufs=3) as attnd,
            tc.tile_pool(name="small", bufs=2) as small,
            tc.tile_pool(name="outp", bufs=4) as outp,
        ):
            # ---- constants ----
            bqt = const.tile([128, HEC], F32, tag="bqt")
            nc.sync.dma_start(bqt[:], bq_ext.rearrange("(c p) -> p c", p=128))
            bqs = const.tile([128, HEC], F32, tag="bqs")
            nc.vector.tensor_scalar_mul(bqs[:], bqt[:], float(SCALE))
            bkt = const.tile([128, HEC], F32, tag="bkt")
            nc.sync.dma_start(bkt[:], bk_ext.rearrange("(c p) -> p c", p=128))

            bv_row = const.tile([1, HE], F32, tag="bv_row")
            nc.sync.dma_start(bv_row[:],
                              bv_ext.rearrange("(o he) -> o he", o=1))
            bv_bc = const.tile([128, HE], F32, tag="bv_bc")
            nc.gpsimd.partition_broadcast(bv_bc[:], bv_row[:], channels=128)

            bo_row = const.tile([1, OUT_D], F32, tag="bo_row")
            nc.sync.dma_start(bo_row[:],
                              bo_ext.rearrange("(o d) -> o d", o=1))
            bo_bc = const.tile([128, OUT_D], F32, tag="bo_bc")
            nc.gpsimd.partition_broadcast(bo_bc[:], bo_row[:], channels=128)

            ones_t = const.tile([128, NH], F32, tag="ones_t")
            nc.vector.memset(ones_t[:], 1.0)

            # ---- phase 1: q/k projections -> qT/kT [e, l] head-pair tiles
            qT_sb = [acts.tile([128, L], MDT, tag=f"qT{i}", name=f"qT_sb{i}")
                     for i in range(HEC)]
            kT_sb = [acts.tile([128, L], MDT, tag=f"kT{i}", name=f"kT_sb{i}")
                     for i in range(HEC)]

            pp_ctx = tc.tile_pool(name="pp", bufs=min(NLC + 2, 6),
                                  space="PSUM")
            pp = pp_ctx.__enter__()

            def load_w(w_ext):
                wt = []
                for dt in range(KC):
                    w = wsl.tile([128, HE], MDT, tag="wsl", name="w_t")
                    nc.sync.dma_start(w[:],
                                      w_ext[dt * 128:(dt + 1) * 128, :])
                    wt.append(w)
                return wt

            def load_xin(in_ext):
                # full-row d-tiles: one big DMA each, live for the phase
                tiles = []
                for dt in range(KC):
                    x = qin.tile([128, L], MDT, tag=f"xin{dt}",
                                 name=f"xin{dt}", bufs=1)
                    nc.sync.dma_start(x[:],
                                      in_ext[dt * 128:(dt + 1) * 128, :])
                    tiles.append(x)
                return tiles

            # ---- v projection first (A@V consumes every s-tile of v)
            wvt = load_w(wv_ext)
            vin = load_xin(vT_ext)
            v_aug = []
            for st in range(NST):
                v = acts.tile([128, NH * (E + 1)], MDT, tag=f"vaug{st}",
                              name=f"vaug{st}")
                v_aug.append(v)
                nc.vector.tensor_copy(
                    v.rearrange("p (h u) -> p h u", u=E + 1)[:, :, E:E + 1],
                    ones_t.rearrange("p (h o) -> p h o", o=1))
            # two interleaved accumulation chains hide the PSUM RAW latency
            for sp in range(NST // 2):
                ps2 = [pp.tile([128, HE], F32, tag="pp", name=f"psv{i}")
                       for i in range(2)]
                for dt in range(KC):
                    for i in range(2):
                        st = 2 * sp + i
                        nc.tensor.matmul(
                            ps2[i][:],
                            vin[dt][:, st * 128:(st + 1) * 128],
                            wvt[dt][:],
                            start=(dt == 0), stop=(dt == KC - 1))
                for i in range(2):
                    st = 2 * sp + i
                    v = v_aug[st]
                    nc.vector.tensor_add(
                        v.rearrange("p (h u) -> p h u", u=E + 1)[:, :, 0:E],
                        ps2[i].rearrange("p (h e) -> p h e", e=E)[:],
                        bv_bc.rearrange("p (h e) -> p h e", e=E)[:])

            # ---- k then q projections; stationary weight slice serves all
            # NLC moving chunks (amortizes LDWEIGHTS 4x)
            for which, w_ext, in_ext, dest in (
                ("k", wk_ext, kT_ext, kT_sb),
                ("q", wq_ext, qT_ext, qT_sb),
            ):
                wt = load_w(w_ext)
                xin = load_xin(in_ext)
                for ec in range(HEC):
                    ps = [pp.tile([128, LCH], F32, tag="pp", name=f"pp{i}")
                          for i in range(NLC)]
                    for dt in range(KC):
                        for lc in range(NLC):
                            nc.tensor.matmul(
                                ps[lc][:],
                                wt[dt][:, ec * 128:(ec + 1) * 128],
                                xin[dt][:, lc * LCH:(lc + 1) * LCH],
                                start=(dt == 0), stop=(dt == KC - 1))
                    for lc in range(NLC):
                        dst = dest[ec][:, lc * LCH:(lc + 1) * LCH]
                        if which == "q":
                            # (psum + bq) * scale
                            nc.vector.tensor_scalar(
                                dst, ps[lc][:], float(SCALE),
                                bqs[:, ec:ec + 1], MULT, ADD)
                        else:
                            nc.vector.tensor_scalar(
                                dst, ps[lc][:], bkt[:, ec:ec + 1], None, ADD)

            # Wo preload goes after the projection-input DMAs in trace order
            # so it doesn't delay the first matmuls (it's only needed from
            # the first output projection onwards)
            wo_sb = []
            for t in range(JE // 128):
                w = const.tile([128, OUT_D], MDT, tag=f"wo{t}",
                               name=f"wo_sb{t}")
                nc.sync.dma_start(w[:], wo_ext[t * 128:(t + 1) * 128, :])
                wo_sb.append(w)

            pp_ctx.__exit__(None, None, None)

            if taps:
                for i in range(HEC):
                    nc.sync.dma_start(dbg_qT[i * 128:(i + 1) * 128, :],
                                      qT_sb[i][:])
                    nc.sync.dma_start(dbg_kT[i * 128:(i + 1) * 128, :],
                                      kT_sb[i][:])
                for st in range(NST):
                    nc.sync.dma_start(dbg_v[st * 128:(st + 1) * 128, :],
                                      v_aug[st][:])

            # ---- phase 2+3: attention + output projection per head pair
            # l-chunks are processed G at a time sharing one wide scores
            # psum tile so the exp runs on G*LCH columns per instruction
            G = 2 if NLC % 2 == 0 else 1
            with (
                tc.tile_pool(name="psc", bufs=2, space="PSUM") as psc,
                tc.tile_pool(name="pav", bufs=2, space="PSUM") as pav,
                tc.tile_pool(name="pout", bufs=2, space="PSUM") as pout,
            ):
                for hp in range(NHP):
                    dups = []
                    for loc in range(2):  # head A (partitions 0:64), head B
                        p0 = loc * 64
                        h = 2 * hp + loc
                        dup = attnd.tile([128, L], MDT, tag="attnd",
                                         name="dup")
                        dups.append(dup)
                        pcps = []
                        for lcw in range(NLC // G):
                            lcs = [lcw * G + g for g in range(G)]
                            pavs = [pav.tile([65, LCH], F32, tag="pav",
                                             name="pavx") for _ in lcs]
                            for st in range(NST):
                                sc = psc.tile([128, G * LCH], F32,
                                              tag="psc", name="sc")
                                for g, lc in enumerate(lcs):
                                    nc.tensor.matmul(
                                        sc[:, g * LCH:(g + 1) * LCH],
                                        kT_sb[hp][p0:p0 + 64,
                                                  st * 128:(st + 1) * 128],
                                        qT_sb[hp][p0:p0 + 64,
                                                  lc * LCH:(lc + 1) * LCH],
                                        start=True, stop=True)
                                ex = expp.tile([128, G * LCH], MDT,
                                               tag="exp", name="ex")
                                nc.scalar.activation(ex[:], sc[:], EXP)
                                if taps and hp == 0 and loc == 0 \
                                        and lcw == 0 and st == 0:
                                    nc.sync.dma_start(dbg_ex[:],
                                                      ex[:, 0:LCH])
                                for g in range(G):
                                    nc.tensor.matmul(
                                        pavs[g][:],
                                        v_aug[st][:, h * (E + 1):
                                                  (h + 1) * (E + 1)],
                                        ex[:, g * LCH:(g + 1) * LCH],
                                        start=(st == 0),
                                        stop=(st == NST - 1))
                            for g in range(G):
                                # quick-release copy frees the PSUM bank
                                # immediately; the (slow) reciprocal runs
                                # once per head, off the accumulate path
                                pcp = small.tile([65, LCH], F32, tag="pcp",
                                                 name="pcp", bufs=NLC)
                                nc.vector.tensor_copy(pcp[:], pavs[g][:])
                                pcps.append(pcp)
                        # softmax epilogue for the whole head, after all
                        # quick-release copies so PSUM banks free promptly
                        for lc in range(NLC):
                            rc = small.tile([1, LCH], F32, tag="rc",
                                            name="rc", bufs=4)
                            nc.vector.reciprocal(rc[:], pcps[lc][64:65, :])
                            bc = small.tile([64, LCH], F32, tag="bc",
                                            name="bc", bufs=4)
                            nc.gpsimd.partition_broadcast(bc[:], rc[:],
                                                          channels=64)
                            nc.vector.tensor_mul(
                                dup[0:64, lc * LCH:(lc + 1) * LCH],
                                pcps[lc][0:64, :], bc[:])
                        # shifted self-copy: partition 64+e, col l holds
                        # attn[e, l+1]; a stride-16 AP at offset 2t then
                        # reads the (2t, 2t+1) j-pair as one K=128
                        # stationary operand for the output projection
                        nc.sync.dma_start(dup[64:128, 0:L - 1],
                                          dup[0:64, 1:L])
                        if taps and hp == 0 and loc == 0:
                            nc.sync.dma_start(dbg_dup[:, 0:L - 1],
                                              dup[:, 0:L - 1])

                    for loc in range(2):
                        h = 2 * hp + loc
                        dup = dups[loc]
                        lhs = dup.rearrange("p (r j) -> p j r", j=J)
                        for dc in range(NDC):
                            po = pout.tile([R, DCH], F32, tag="pout",
                                           name="po")
                            for t in range(JE // 128):
                                nc.tensor.matmul(
                                    po[:],
                                    lhs[:, 2 * t, :],
                                    wo_sb[t][:, dc * DCH:(dc + 1) * DCH],
                                    start=(t == 0),
                                    stop=(t == JE // 128 - 1))
                            ob = outp.tile([R, DCH], F32, tag="outp",
                                           name="ob")
                            nc.vector.tensor_add(
                                ob[:], po[:],
                                bo_bc[0:R, dc * DCH:(dc + 1) * DCH])
                            nc.sync.dma_start(
                                out_ext[h * R:(h + 1) * R,
                                        dc * DCH:(dc + 1) * DCH],
                                ob[:])

    nc.compile()
    return nc


# ---------------------------------------------------------------------------
# host side
# ---------------------------------------------------------------------------

_NC_CACHE = {}

FULL_KEY = (2048, 1024, 8, 1024, "bf16")


def _get_nc(key=FULL_KEY):
    if key not in _NC_CACHE:
        _NC_CACHE[key] = build_core_kernel(*key)
    return _NC_CACHE[key]


def _np_mm_dtype(mm_dt):
    if mm_dt == "bf16":
        import ml_dtypes
        return ml_dtypes.bfloat16
    return np.float32


def make_in_maps(queries, keys, values, Wq, bq, Wk, bk, Wv, bv, Wo, bo,
                 mm_dt="f32r"):
    """Shard: core c handles batch c//2, heads NH*(c%2) .. NH*(c%2)+NH."""
    f = np.float32
    md = _np_mm_dtype(mm_dt)
    half_w = np.asarray(Wq).shape[1] // 2
    in_maps = []
    for c in range(8):
        b, half = c // 2, c % 2
        cs = slice(half * half_w, (half + 1) * half_w)
        in_maps.append({
            "qT": np.ascontiguousarray(np.asarray(queries[b], f).T.astype(md)),
            "kT": np.ascontiguousarray(np.asarray(keys[b], f).T.astype(md)),
            "vT": np.ascontiguousarray(np.asarray(values[b], f).T.astype(md)),
            "wq": np.ascontiguousarray(np.asarray(Wq, f)[:, cs].astype(md)),
            "wk": np.ascontiguousarray(np.asarray(Wk, f)[:, cs].astype(md)),
            "wv": np.ascontiguousarray(np.asarray(Wv, f)[:, cs].astype(md)),
            "bq": np.ascontiguousarray(np.asarray(bq, f)[cs]),
            "bk": np.ascontiguousarray(np.asarray(bk, f)[cs]),
            "bv": np.ascontiguousarray(np.asarray(bv, f)[cs]),
            "wo": np.ascontiguousarray(np.asarray(Wo, f).astype(md)),
            "bo": np.ascontiguousarray(np.asarray(bo, f)),
        })
    return in_maps


def assemble_output(results, B=4, L=2048, OUT_D=1024):
    out = np.empty((B, L, OUT_D), np.float32)
    half_rows = L // 2
    for c in range(8):
        b, half = c // 2, c % 2
        out[b, half * half_rows:(half + 1) * half_rows, :] = results[c]["out"]
    return out


def run_on_hw(inputs, trace=False, key=FULL_KEY, **kw):
    nc = _get_nc(key)
    in_maps = make_in_maps(**inputs, mm_dt=key[4])
    res = run_bass_kernel_spmd(nc, in_maps, core_ids=list(range(8)),
                               trace=trace, **kw)
    return assemble_output(res.results), res


def kernel(**inputs) -> np.ndarray:
    out, _ = run_on_hw(inputs, trace=False)
    return out


# revision 29
# speedup vs baseline: 1.9066x; 1.1545x over previous
"""Trainium2 Bass kernel for the nn_AttentionLayer problem.

Full multi-head attention layer, B=4, L=S=2048, d_model=1024, 16 heads of
dim 64, with the reference's "mix=True" transpose-then-flatten before the
output projection.

Key observation: the mix reshape means output row l' = h*128 + l//16 of each
batch depends ONLY on head h.  So sharding 8 cores as (batch, half-of-heads)
makes every core produce a disjoint, contiguous 1024-row slice of the output
with zero cross-core communication.

Per-core dataflow:
  - inputs arrive host-transposed ([D, L]) so the QKV projections contract
    d_model on the partition dim with natural weight layouts
  - q/k projections produce qT/kT [e, l] (heads on partitions, pairs of
    heads per 128-partition tile), with 1/sqrt(64) and bias folded in
  - scoresT tile [s=128, l=512] = kT.T @ qT per (s-tile, l-chunk); exp on
    the scalar engine; A@V accumulates v_aug = [v | ones] stationary so
    row 64 of the accumulator is the softmax denominator
  - normalize with vector-reciprocal + gpsimd partition_broadcast
  - the normalized attn [64, L] is self-copied (SBUF->SBUF DMA) to
    partitions 64..127 shifted by one position, which makes the output
    projection a clean K=128 matmul against natural Wo row-pair tiles

All matmuls run in `mm_dt` (float32r by default: full-rate fp32 on the PE).
"""

import numpy as np

import concourse.bass as bass
import concourse.mybir as mybir
import concourse.tile as tile
from concourse import bacc
from concourse.bass_utils import run_bass_kernel_spmd

F32 = mybir.dt.float32
EXP = mybir.ActivationFunctionType.Exp
MULT = mybir.AluOpType.mult
ADD = mybir.AluOpType.add

E = 64          # head dim
J = 16          # mix factor: total heads in the reference model
JE = J * E      # 1024 rows of Wo

MM_DTS = {
    "f32r": mybir.dt.float32r,
    "f32": mybir.dt.float32,
    "bf16": mybir.dt.bfloat16,
}


def build_core_kernel(L=2048, D=1024, NH=8, OUT_D=1024, mm_dt="f32r",
                      taps=False):
    """Builds the per-core Bacc graph (SPMD: all 8 cores run this)."""
    HE = NH * E               # projected width per core
    HEC = HE // 128           # qT/kT tiles (head pairs)
    NHP = NH // 2             # head pairs
    KC = D // 128             # contraction tiles for projections
    R = L // J                # output rows per head
    LCH = min(512, L)         # l-chunk
    NLC = L // LCH
    NST = L // 128            # s-tiles
    DCH = min(512, OUT_D)     # out-proj n-chunk
    NDC = OUT_D // DCH
    SCALE = 1.0 / np.sqrt(E)
    assert L % J == 0 and R <= 128 and HE % 128 == 0

    MDT = MM_DTS[mm_dt]

    nc = bacc.Bacc("TRN2", target_bir_lowering=False, debug=False,
                   enable_asserts=False)

    qT_ext = nc.declare_dram_parameter("qT", [D, L], MDT, isOutput=False)
    kT_ext = nc.declare_dram_parameter("kT", [D, L], MDT, isOutput=False)
    vT_ext = nc.declare_dram_parameter("vT", [D, L], MDT, isOutput=False)
    wq_ext = nc.declare_dram_parameter("wq", [D, HE], MDT, isOutput=False)
    wk_ext = nc.declare_dram_parameter("wk", [D, HE], MDT, isOutput=False)
    wv_ext = nc.declare_dram_parameter("wv", [D, HE], MDT, isOutput=False)
    bq_ext = nc.declare_dram_parameter("bq", [HE], F32, isOutput=False)
    bk_ext = nc.declare_dram_parameter("bk", [HE], F32, isOutput=False)
    bv_ext = nc.declare_dram_parameter("bv", [HE], F32, isOutput=False)
    wo_ext = nc.declare_dram_parameter("wo", [JE, OUT_D], MDT, isOutput=False)
    bo_ext = nc.declare_dram_parameter("bo", [OUT_D], F32, isOutput=False)
    out_ext = nc.declare_dram_parameter("out", [NH * R, OUT_D], F32,
                                        isOutput=True)
    HEC_ = HE // 128
    if taps:
        dbg_qT = nc.declare_dram_parameter("dbg_qT", [HEC_ * 128, L], MDT,
                                           isOutput=True)
        dbg_kT = nc.declare_dram_parameter("dbg_kT", [HEC_ * 128, L], MDT,
                                           isOutput=True)
        dbg_v = nc.declare_dram_parameter("dbg_v", [(L // 128) * 128,
                                                    NH * (E + 1)], MDT,
                                          isOutput=True)
        dbg_ex = nc.declare_dram_parameter("dbg_ex", [128, min(512, L)], MDT,
                                           isOutput=True)
        dbg_dup = nc.declare_dram_parameter("dbg_dup", [128, L], MDT,
                                            isOutput=True)

    with tile.TileContext(nc) as tc:
        with (
            tc.tile_pool(name="const", bufs=1) as const,
            tc.tile_pool(name="wsl", bufs=min(KC + 1, 3 * KC)) as wsl,
            tc.tile_pool(name="qin", bufs=3) as qin,
            tc.tile_pool(name="acts", bufs=1) as acts,
            tc.tile_pool(name="expp", bufs=6) as expp,
            tc.tile_pool(name="attnd", bufs=3) as attnd,
            tc.tile_pool(name="small", bufs=2) as small,
            tc.tile_pool(name="outp", bufs=4) as outp,
        ):
            # ---- constants ----
            bqt = const.tile([128, HEC], F32, tag="bqt")
            nc.sync.dma_start(bqt[:], bq_ext.rearrange("(c p) -> p c", p=128))
            bqs = const.tile([128, HEC], F32, tag="bqs")
            nc.vector.tensor_scalar_mul(bqs[:], bqt[:], float(SCALE))
            bkt = const.tile([128, HEC], F32, tag="bkt")
            nc.sync.dma_start(bkt[:], bk_ext.rearrange("(c p) -> p c", p=128))

            bv_row = const.tile([1, HE], F32, tag="bv_row")
            nc.sync.dma_start(bv_row[:],
                              bv_ext.rearrange("(o he) -> o he", o=1))
            bv_bc = const.tile([128, HE], F32, tag="bv_bc")
            nc.gpsimd.partition_broadcast(bv_bc[:], bv_row[:], channels=128)

            bo_row = const.tile([1, OUT_D], F32, tag="bo_row")
            nc.sync.dma_start(bo_row[:],
                              bo_ext.rearrange("(o d) -> o d", o=1))
            bo_bc = const.tile([128, OUT_D], F32, tag="bo_bc")
            nc.gpsimd.partition_broadcast(bo_bc[:], bo_row[:], channels=128)

            ones_t = const.tile([128, NH], F32, tag="ones_t")
            nc.vector.memset(ones_t[:], 1.0)

            # ---- phase 1: q/k projections -> qT/kT [e, l] head-pair tiles
            qT_sb = [acts.tile([128, L], MDT, tag=f"qT{i}", name=f"qT_sb{i}")
                     for i in range(HEC)]
            kT_sb = [acts.tile([128, L], MDT, tag=f"kT{i}", name=f"kT_sb{i}")
                     for i in range(HEC)]

            pp_ctx = tc.tile_pool(name="pp", bufs=min(NLC + 2, 6),
                                  space="PSUM")
            pp = pp_ctx.__enter__()

            def load_w(w_ext):
                wt = []
                for dt in range(KC):
                    w = wsl.tile([128, HE], MDT, tag="wsl", name="w_t")
                    nc.sync.dma_start(w[:],
                                      w_ext[dt * 128:(dt + 1) * 128, :])
                    wt.append(w)
                return wt

            def load_xin(in_ext):
                # full-row d-tiles: one big DMA each, live for the phase
                tiles = []
                for dt in range(KC):
                    x = qin.tile([128, L], MDT, tag=f"xin{dt}",
                                 name=f"xin{dt}", bufs=1)
                    nc.sync.dma_start(x[:],
                                      in_ext[dt * 128:(dt + 1) * 128, :])
                    tiles.append(x)
                return tiles

            # ---- v projection first (A@V consumes every s-tile of v)
            wvt = load_w(wv_ext)
            vin = load_xin(vT_ext)
            v_aug = []
            for st in range(NST):
                v = acts.tile([128, NH * (E + 1)], MDT, tag=f"vaug{st}",
                              name=f"vaug{st}")
                v_aug.append(v)
                nc.vector.tensor_copy(
                    v.rearrange("p (h u) -> p h u", u=E + 1)[:, :, E:E + 1],
                    ones_t.rearrange("p (h o) -> p h o", o=1))
            # two interleaved accumulation chains hide the PSUM RAW latency
            for sp in range(NST // 2):
                ps2 = [pp.tile([128, HE], F32, tag="pp", name=f"psv{i}")
                       for i in range(2)]
                for dt in range(KC):
                    for i in range(2):
                        st = 2 * sp + i
                        nc.tensor.matmul(
                            ps2[i][:],
                            vin[dt][:, st * 128:(st + 1) * 128],
                            wvt[dt][:],
                            start=(dt == 0), stop=(dt == KC - 1))
                for i in range(2):
                    st = 2 * sp + i
                    v = v_aug[st]
                    nc.vector.tensor_add(
                        v.rearrange("p (h u) -> p h u", u=E + 1)[:, :, 0:E],
                        ps2[i].rearrange("p (h e) -> p h e", e=E)[:],
                        bv_bc.rearrange("p (h e) -> p h e", e=E)[:])

            # ---- k then q projections; stationary weight slice serves all
            # NLC moving chunks (amortizes LDWEIGHTS 4x)
            for which, w_ext, in_ext, dest in (
                ("k", wk_ext, kT_ext, kT_sb),
                ("q", wq_ext, qT_ext, qT_sb),
            ):
                wt = load_w(w_ext)
                xin = load_xin(in_ext)
                for ec in range(HEC):
                    ps = [pp.tile([128, LCH], F32, tag="pp", name=f"pp{i}")
                          for i in range(NLC)]
                    for dt in range(KC):
                        for lc in range(NLC):
                            nc.tensor.matmul(
                                ps[lc][:],
                                wt[dt][:, ec * 128:(ec + 1) * 128],
                                xin[dt][:, lc * LCH:(lc + 1) * LCH],
                                start=(dt == 0), stop=(dt == KC - 1))
                    for lc in range(NLC):
                        dst = dest[ec][:, lc * LCH:(lc + 1) * LCH]
                        if which == "q":
                            # (psum + bq) * scale
                            nc.vector.tensor_scalar(
                                dst, ps[lc][:], float(SCALE),
                                bqs[:, ec:ec + 1], MULT, ADD)
                        else:
                            nc.vector.tensor_scalar(
                                dst, ps[lc][:], bkt[:, ec:ec + 1], None, ADD)

            # Wo preload goes after the projection-input DMAs in trace order
            # so it doesn't delay the first matmuls (it's only needed from
            # the first output projection onwards)
            wo_sb = []
            for t in range(JE // 128):
                w = const.tile([128, OUT_D], MDT, tag=f"wo{t}",
                               name=f"wo_sb{t}")
                nc.sync.dma_start(w[:], wo_ext[t * 128:(t + 1) * 128, :])
                wo_sb.append(w)

            pp_ctx.__exit__(None, None, None)

            if taps:
                for i in range(HEC):
                    nc.sync.dma_start(dbg_qT[i * 128:(i + 1) * 128, :],
                                      qT_sb[i][:])
                    nc.sync.dma_start(dbg_kT[i * 128:(i + 1) * 128, :],
                                      kT_sb[i][:])
                for st in range(NST):
                    nc.sync.dma_start(dbg_v[st * 128:(st + 1) * 128, :],
                                      v_aug[st][:])

            # ---- phase 2+3: attention + output projection per head pair
            # Both heads of a pair are processed together: their scores land
            # in one wide PSUM tile via two matmuls on complementary PE
            # row-groups (true row-packing), one wide exp covers both, and
            # each head's A@V accumulates separately.  The output projection
            # of pair p is emitted inside pair p+1's stream so the PE fills
            # ACT-bound slack instead of starving the scalar engine.
            W = 2 * LCH
            with (
                tc.tile_pool(name="psc", bufs=2, space="PSUM") as psc,
                tc.tile_pool(name="pacc", bufs=4, space="PSUM") as pacc,
            ):
                def emit_outproj(dups, hp):
                    for loc in range(2):
                        h = 2 * hp + loc
                        lhs = dups[loc].rearrange("p (r j) -> p j r", j=J)
                        for dc in range(NDC):
                            po = pacc.tile([R, DCH], F32, tag="acc",
                                           name="po")
                            for t in range(JE // 128):
                                nc.tensor.matmul(
                                    po[:],
                                    lhs[:, 2 * t, :],
                                    wo_sb[t][:, dc * DCH:(dc + 1) * DCH],
                                    start=(t == 0),
                                    stop=(t == JE // 128 - 1))
                            ob = outp.tile([R, DCH], F32, tag="outp",
                                           name="ob")
                            nc.vector.tensor_add(
                                ob[:], po[:],
                                bo_bc[0:R, dc * DCH:(dc + 1) * DCH])
                            nc.sync.dma_start(
                                out_ext[h * R:(h + 1) * R,
                                        dc * DCH:(dc + 1) * DCH],
                                ob[:])

                pending = None
                for hp in range(NHP):
                    dups = [attnd.tile([128, L], MDT, tag="attnd",
                                       name="dup") for _ in range(2)]
                    pcps = [[], []]
                    for lc in range(NLC):
                        pavx = [pacc.tile([65, LCH], F32, tag="acc",
                                          name="pavx") for _ in range(2)]
                        for st in range(NST):
                            sc = psc.tile([128, W], F32, tag="psc",
                                          name="sc")
                            for loc in range(2):
                                p0 = loc * 64
                                nc.tensor.matmul(
                                    sc[:, loc * LCH:(loc + 1) * LCH],
                                    kT_sb[hp][p0:p0 + 64,
                                              st * 128:(st + 1) * 128],
                                    qT_sb[hp][p0:p0 + 64,
                                              lc * LCH:(lc + 1) * LCH],
                                    start=True, stop=True)
                            ex = expp.tile([128, W], MDT, tag="exp",
                                           name="ex")
                            nc.scalar.activation(ex[:], sc[:], EXP)
                            if taps and hp == 0 and lc == 0 and st == 0:
                                nc.sync.dma_start(dbg_ex[:], ex[:, 0:LCH])
                            for loc in range(2):
                                h = 2 * hp + loc
                                nc.tensor.matmul(
                                    pavx[loc][:],
                                    v_aug[st][:, h * (E + 1):
                                              (h + 1) * (E + 1)],
                                    ex[:, loc * LCH:(loc + 1) * LCH],
                                    start=(st == 0), stop=(st == NST - 1))
                        for loc in range(2):
                            pcp = small.tile([65, LCH], F32, tag="pcp",
                                             name="pcp", bufs=2 * NLC)
                            nc.vector.tensor_copy(pcp[:], pavx[loc][:])
                            pcps[loc].append(pcp)
                        if lc == 0 and pending is not None:
                            emit_outproj(*pending)
                            pending = None
                    for loc in range(2):
                        dup = dups[loc]
                        for lc in range(NLC):
                            rc = small.tile([1, LCH], F32, tag="rc",
                                            name="rc", bufs=4)
                            nc.vector.reciprocal(rc[:],
                                                 pcps[loc][lc][64:65, :])
                            bc = small.tile([64, LCH], F32, tag="bc",
                                            name="bc", bufs=4)
                            nc.gpsimd.partition_broadcast(bc[:], rc[:],
                                                          channels=64)
                            nc.vector.tensor_mul(
                                dup[0:64, lc * LCH:(lc + 1) * LCH],
                                pcps[loc][lc][0:64, :], bc[:])
                        nc.sync.dma_start(dup[64:128, 0:L - 1],
                                          dup[0:64, 1:L])
                        if taps and hp == 0 and loc == 0:
                            nc.sync.dma_start(dbg_dup[:, 0:L - 1],
                                              dup[:, 0:L - 1])
                    if pending is not None:
                        emit_outproj(*pending)
                    pending = (dups, hp)
                emit_outproj(*pending)

    nc.compile()
    return nc


# ---------------------------------------------------------------------------
# host side
# ---------------------------------------------------------------------------

_NC_CACHE = {}

FULL_KEY = (2048, 1024, 8, 1024, "bf16")


def _get_nc(key=FULL_KEY):
    if key not in _NC_CACHE:
        _NC_CACHE[key] = build_core_kernel(*key)
    return _NC_CACHE[key]


def _np_mm_dtype(mm_dt):
    if mm_dt == "bf16":
        import ml_dtypes
        return ml_dtypes.bfloat16
    return np.float32


def make_in_maps(queries, keys, values, Wq, bq, Wk, bk, Wv, bv, Wo, bo,
                 mm_dt="f32r"):
    """Shard: core c handles batch c//2, heads NH*(c%2) .. NH*(c%2)+NH."""
    f = np.float32
    md = _np_mm_dtype(mm_dt)
    half_w = np.asarray(Wq).shape[1] // 2
    in_maps = []
    for c in range(8):
        b, half = c // 2, c % 2
        cs = slice(half * half_w, (half + 1) * half_w)
        in_maps.append({
            "qT": np.ascontiguousarray(np.asarray(queries[b], f).T.astype(md)),
            "kT": np.ascontiguousarray(np.asarray(keys[b], f).T.astype(md)),
            "vT": np.ascontiguousarray(np.asarray(values[b], f).T.astype(md)),
            "wq": np.ascontiguousarray(np.asarray(Wq, f)[:, cs].astype(md)),
            "wk": np.ascontiguousarray(np.asarray(Wk, f)[:, cs].astype(md)),
            "wv": np.ascontiguousarray(np.asarray(Wv, f)[:, cs].astype(md)),
            "bq": np.ascontiguousarray(np.asarray(bq, f)[cs]),
            "bk": np.ascontiguousarray(np.asarray(bk, f)[cs]),
            "bv": np.ascontiguousarray(np.asarray(bv, f)[cs]),
            "wo": np.ascontiguousarray(np.asarray(Wo, f).astype(md)),
            "bo": np.ascontiguousarray(np.asarray(bo, f)),
        })
    return in_maps


def assemble_output(results, B=4, L=2048, OUT_D=1024):
    out = np.empty((B, L, OUT_D), np.float32)
    half_rows = L // 2
    for c in range(8):
        b, half = c // 2, c % 2
        out[b, half * half_rows:(half + 1) * half_rows, :] = results[c]["out"]
    return out


def run_on_hw(inputs, trace=False, key=FULL_KEY, **kw):
    nc = _get_nc(key)
    in_maps = make_in_maps(**inputs, mm_dt=key[4])
    res = run_bass_kernel_spmd(nc, in_maps, core_ids=list(range(8)),
                               trace=trace, **kw)
    return assemble_output(res.results), res


def kernel(**inputs) -> np.ndarray:
    out, _ = run_on_hw(inputs, trace=False)
    return out
